# revision 1
# baseline (speedup 1.0000x reference)
"""Trainium2 Bass kernel for CausalSelfAttention (RoPE + ALiBi + causal mask).

Sharding: 16 heads tensor-parallel across 8 NeuronCores (2 heads/core).
Per core:
  phase 1: qkv projection from replicated x^T in bf16 (halves the DMA
           stream; PSUM accumulation stays f32). RoPE applied on the fly
           via cross-partition DVE multiplies against a sign-folded sin
           table (no PE rotation matmul). q^T,k^T kept in SBUF [d, t].
  phase 2: attention per (batch, head) in transposed layout
           S^T[j, i] = k^T.T @ q^T; ALiBi+mask added by DVE into SBUF
           (frees the PSUM score bank after one op); exp on ScalarE;
           row-sums via ones-matmul into a shared per-(b,icx) PSUM bank;
           y^T accumulated on TensorE; reciprocal broadcast on GpSimd.
           Diagonal-block offsets are clamped to keep matmul free dims
           >= 256 (f32r runs 4x slower below that); over-computed columns
           hit an all-NEG bias slot and exp to 0.
  phase 3: out partial = y @ W_proj (rows of the core's heads),
           interleaved with phase 2 per query chunk.
Host: sums the 8 partial outputs.

Attention matmuls run in float32r (TF32-like, full PE rate at free dim
>= 256); the qkv projection runs in bf16. DMA issue order is consumption
order so the first chunk's weights/x lead and phase-2/3 constants trail.
"""

import math
from contextlib import ExitStack

import numpy as np

import concourse.bass as bass
import concourse.mybir as mybir
import concourse.tile as tile
from concourse import bacc
from concourse.bass_utils import run_bass_kernel_spmd

B, T, DM = 2, 2048, 2048
H, HD = 16, 128
ROWS = B * T                      # 4096
NCORES = 8
HPC = H // NCORES                 # 2 heads per core
ROPE_THETA = 10000.0
SQHD = math.sqrt(HD)
M_OFF = 18.0                      # softmax stability offset
NEG = -1.0e30

TCH = 512                         # t-chunk width in phase 1
NCH = ROWS // TCH                 # 8
CT = DM // 128                    # 16 contraction tiles
NT = T // 128                     # 16 key/query tiles per batch
IC = 512                          # query chunk in phase 2
NIC = T // IC                     # 4

F32 = mybir.dt.float32
F32R = mybir.dt.float32r
BF16 = mybir.dt.bfloat16
MULT = mybir.AluOpType.mult
ADD = mybir.AluOpType.add
EXP = mybir.ActivationFunctionType.Exp


def build_program(phases="123", loop_n=1):
    nc = bacc.Bacc("TRN2", target_bir_lowering=False, debug=False,
                   num_devices=NCORES)
    xT = nc.dram_tensor("xT", [DM, ROWS], BF16, kind="ExternalInput").ap()
    wq = nc.dram_tensor("wq", [DM, HPC * HD], BF16, kind="ExternalInput").ap()
    wk = nc.dram_tensor("wk", [DM, HPC * HD], BF16, kind="ExternalInput").ap()
    wv = nc.dram_tensor("wv", [DM, HPC * HD], BF16, kind="ExternalInput").ap()
    wp = nc.dram_tensor("wp", [HPC * HD, DM], F32, kind="ExternalInput").ap()
    cosT = nc.dram_tensor("cosT", [128, T], F32, kind="ExternalInput").ap()
    sinT = nc.dram_tensor("sinT", [128, T], F32, kind="ExternalInput").ap()
    biasd = nc.dram_tensor("biasd", [128, HPC, 17, 128], F32,
                           kind="ExternalInput").ap()
    ones128 = nc.dram_tensor("ones128", [128, 1], F32, kind="ExternalInput").ap()
    out = nc.dram_tensor("out", [ROWS, DM], F32, kind="ExternalOutput").ap()

    xT3 = xT.rearrange("(o p) t -> p o t", p=128)

    with tile.TileContext(nc) as tc, ExitStack() as ctx:
        const = ctx.enter_context(tc.tile_pool(name="const", bufs=1))
        qkp = ctx.enter_context(tc.tile_pool(name="qk", bufs=1))

        q_sb = [qkp.tile([128, ROWS], F32R, tag=f"q{e}", name=f"q{e}")
                for e in range(HPC)]
        k_sb = [qkp.tile([128, ROWS], F32R, tag=f"k{e}", name=f"k{e}")
                for e in range(HPC)]
        v_keep = qkp.tile([128, B * NT, HPC * HD], F32R, tag="vk", name="vk")

        cos_sb = const.tile([128, T], F32, tag="cos")
        sin_sb = const.tile([128, T], F32, tag="sin")
        ones128_sb = const.tile([128, 1], F32R, tag="o128")
        bias_sb = const.tile([128, HPC, 17, 128], F32, tag="bias")
        wp_sb = const.tile([128, HPC, DM], F32R, tag="wp")

        if loop_n > 1:
            # timing mode: run the whole body loop_n times on-device
            ctx.enter_context(tc.For_i(0, loop_n, 1))

        # ---------------- phase 1: qkv + rope ----------------
        with tc.tile_pool(name="w1", bufs=1) as wpool, \
             tc.tile_pool(name="xt", bufs=24) as xpool, \
             tc.tile_pool(name="rope", bufs=3) as rpool, \
             tc.tile_pool(name="p1", bufs=6, space="PSUM") as ps1, \
             tc.tile_pool(name="pv", bufs=2, space="PSUM") as psv:
            wq_sb = wpool.tile([128, CT, HPC * HD], BF16, tag="wq")
            wk_sb = wpool.tile([128, CT, HPC * HD], BF16, tag="wk")
            wv_sb = wpool.tile([128, CT, HPC * HD], BF16, tag="wv")
            # DMA issue order is consumption order: everything chunk 0 needs
            # (all weight groups + its 16 x tiles, interleaved) leads; then
            # rope tables; phase-2/3 constants trail behind
            xts0 = []
            for g in range(4):
                cg = slice(g * 4, (g + 1) * 4)
                nc.sync.dma_start(wq_sb[:, cg, :], wq.rearrange("(o p) e -> p o e", p=128)[:, cg, :])
                nc.sync.dma_start(wk_sb[:, cg, :], wk.rearrange("(o p) e -> p o e", p=128)[:, cg, :])
                for ci in range(g * 4, g * 4 + 4):
                    xt0 = xpool.tile([128, TCH], BF16, tag="xt")
                    nc.sync.dma_start(xt0[:], xT3[:, ci, 0:TCH])
                    xts0.append(xt0)
            nc.sync.dma_start(cos_sb[:], cosT)
            nc.sync.dma_start(sin_sb[:], sinT)
            nc.sync.dma_start(ones128_sb[:], ones128.bitcast(F32R))
            for g in range(4):
                cg = slice(g * 4, (g + 1) * 4)
                nc.sync.dma_start(wv_sb[:, cg, :], wv.rearrange("(o p) e -> p o e", p=128)[:, cg, :])
            nc.sync.dma_start(bias_sb[:], biasd)
            nc.sync.dma_start(wp_sb[:], wp.rearrange("(o p) e -> p o e", p=128).bitcast(F32R))

            for tchunk in range(NCH):
                t0 = tchunk * TCH
                if tchunk == 0:
                    xts = xts0
                else:
                    xts = []
                    for ci in range(CT):
                        xt_t = xpool.tile([128, TCH], BF16, tag="xt")
                        nc.sync.dma_start(xt_t[:], xT3[:, ci, t0:t0 + TCH])
                        xts.append(xt_t)

                cs = slice(t0 % T, t0 % T + TCH)
                for dst, w_sb in ((q_sb, wq_sb), (k_sb, wk_sb)):
                    for et in range(HPC):
                        ps_q = ps1.tile([128, TCH], F32, tag="psq")
                        for ci in range(CT):
                            nc.tensor.matmul(ps_q[:],
                                             w_sb[:, ci, et * HD:(et + 1) * HD],
                                             xts[ci][:],
                                             start=(ci == 0), stop=(ci == CT - 1))
                        qraw = rpool.tile([128, TCH], F32R, tag="qraw")
                        nc.scalar.copy(qraw[:], ps_q[:])
                        # rotate-half via cross-partition DVE ops; sin_sb rows
                        # 0:64 hold -sin so tmp = rot(q) * sin in two halves
                        tmp = rpool.tile([128, TCH], F32, tag="tmp")
                        # both INPUTS share a base partition (verifier
                        # requirement); only the output is offset. sin rows
                        # 64:128 hold -sin_h, rows 0:64 hold +sin_h.
                        nc.vector.tensor_mul(tmp[0:64, :], qraw[64:128, :],
                                             sin_sb[64:128, cs])
                        nc.vector.tensor_mul(tmp[64:128, :], qraw[0:64, :],
                                             sin_sb[0:64, cs])
                        dcols = dst[et][:, t0:t0 + TCH]
                        nc.vector.tensor_mul(dcols, qraw[:], cos_sb[:, cs])
                        nc.vector.tensor_add(dcols, dcols, tmp[:])

                for tt in range(TCH // 128):
                    ps_vt = psv.tile([128, HPC * HD], F32, tag="psv")
                    for ci in range(CT):
                        nc.tensor.matmul(ps_vt[:],
                                         xts[ci][:, tt * 128:(tt + 1) * 128],
                                         wv_sb[:, ci, :],
                                         start=(ci == 0), stop=(ci == CT - 1))
                    nc.scalar.copy(v_keep[:, t0 // 128 + tt, :], ps_vt[:])

        # ---------------- phase 2+3: attention + projection ----------------
        do2 = "2" in phases
        with tc.tile_pool(name="wt", bufs=8) as wpool2, \
             tc.tile_pool(name="yb", bufs=2) as ypool, \
             tc.tile_pool(name="sm", bufs=2) as smpool, \
             tc.tile_pool(name="ost", bufs=3) as ostp, \
             tc.tile_pool(name="pssc", bufs=2, space="PSUM") as pssc, \
             tc.tile_pool(name="psacc", bufs=2, space="PSUM") as psacc, \
             tc.tile_pool(name="psm", bufs=2, space="PSUM") as psmisc, \
             tc.tile_pool(name="pso", bufs=2, space="PSUM") as pso:
            for b in range(B if do2 else 0):
                y_b = ypool.tile([128, HPC, T], F32R, tag="yb")
                for icx in range(NIC):
                    i0 = b * T + icx * IC
                    jt_hi = (icx + 1) * (IC // 128)
                    for hi in range(HPC):
                        ps_sum = psmisc.tile([1, IC], F32, tag="psm")
                        ps_y = psacc.tile([128, IC], F32, tag="psy")
                        for jt in range(jt_hi):
                            # skip fully-masked query columns (i-tile >= jt),
                            # clamped to keep free dim >= 256 (f32r full rate);
                            # the over-computed columns hit the all-NEG bias
                            # slot and exp to exactly 0.
                            o = min(max(0, jt * 128 - icx * IC), IC - 256)
                            n = IC - o
                            ps_sc = pssc.tile([128, IC], F32, tag="pssc")
                            nc.tensor.matmul(
                                ps_sc[:, o:],
                                k_sb[hi][:, b * T + jt * 128: b * T + (jt + 1) * 128],
                                q_sb[hi][:, i0 + o:i0 + IC],
                                start=True, stop=True)
                            # bias slot s = (i-tile - jt) + 1; slot 0 = all-NEG
                            d0 = (icx * IC + o) // 128 - jt + 1
                            ps3 = ps_sc[:, o:].rearrange("p (a c) -> p a c", c=128)
                            w_t = wpool2.tile([128, IC], F32R, tag="wt")
                            w3 = w_t[:, o:].rearrange("p (a c) -> p a c", c=128)
                            # biased scores land in SBUF so the PSUM score
                            # bank frees after one DVE op, not after exp
                            nc.vector.scalar_tensor_tensor(
                                out=w3, in0=ps3, scalar=1.0,
                                in1=bias_sb[:, hi, d0:d0 + n // 128, :],
                                op0=MULT, op1=ADD)
                            nc.scalar.activation(w_t[:, o:], w_t[:, o:], EXP,
                                                 bias=0.0, scale=1.0 / SQHD)
                            nc.tensor.matmul(ps_y[:, o:], v_keep[:, b * NT + jt, hi * HD:(hi + 1) * HD], w_t[:, o:],
                                             start=(jt == 0), stop=(jt == jt_hi - 1))
                            nc.tensor.matmul(ps_sum[0:1, o:], ones128_sb[:], w_t[:, o:],
                                             start=(jt == 0), stop=(jt == jt_hi - 1))
                        recip = smpool.tile([1, IC], F32R, tag="recip")
                        with nc.allow_low_precision(reason="f32r is 4-byte"):
                            nc.vector.reciprocal(recip[:], ps_sum[0:1, :])
                        # broadcast recip down 128 partitions on the (idle)
                        # gpsimd engine; frees PE of the ones1 matmul and ACT
                        # of the staging copy
                        bca = smpool.tile([128, IC], F32R, tag="bca")
                        nc.gpsimd.partition_broadcast(bca[:], recip[0:1, :])
                        nc.vector.tensor_mul(y_b[:, hi, icx * IC:(icx + 1) * IC],
                                             ps_y[:], bca[:])

                    if "3" not in phases:
                        continue
                    # projection of this i-chunk's rows (y ready for both heads)
                    for tt in range(icx * (IC // 128), (icx + 1) * (IC // 128)):
                        for ec in range(DM // 512):
                            ps_out = pso.tile([128, 512], F32, tag="pso")
                            for dt_ in range(HPC):
                                nc.tensor.matmul(ps_out[:],
                                                 y_b[:, dt_, tt * 128:(tt + 1) * 128],
                                                 wp_sb[:, dt_, ec * 512:(ec + 1) * 512],
                                                 start=(dt_ == 0), stop=(dt_ == HPC - 1))
                            o_stage = ostp.tile([128, 512], F32, tag="ost")
                            if (tt * 4 + ec) % 2 == 0:
                                nc.vector.tensor_copy(o_stage[:], ps_out[:])
                            else:
                                nc.scalar.copy(o_stage[:], ps_out[:])
                            r0 = b * T + tt * 128
                            nc.sync.dma_start(out[r0:r0 + 128, ec * 512:(ec + 1) * 512],
                                              o_stage[:])

    nc.compile()
    return nc


def _host_tensors():
    """Core-independent constant inputs."""
    inv_freq = 1.0 / (ROPE_THETA ** (np.arange(0, HD, 2, dtype=np.float64) / HD))
    ang = np.arange(T, dtype=np.float64)[:, None] * inv_freq[None, :]   # [T, 64]
    cos_h = np.cos(ang).T.astype(np.float32)                            # [64, T]
    sin_h = np.sin(ang).T.astype(np.float32)
    cosT = np.concatenate([cos_h, cos_h], axis=0)                       # [128, T]
    # tmp[0:64] = q[64:128] * sinT[64:128] needs -sin there; tmp[64:128]
    # = q[0:64] * sinT[0:64] needs +sin (halves hold identical angles)
    sinT = np.concatenate([sin_h, -sin_h], axis=0)

    ones128 = np.ones((128, 1), dtype=np.float32)
    return cosT, sinT, ones128


def _bias_tiles(h0):
    """[128, HPC, 17, 128] additive pre-scale bias, slot s = (it - jt) + 1.

    Slot 0 (it < jt, fully masked) is all NEG; slot 1 (diagonal) has the
    upper triangle NEG; slots 2.. are pure sqrt(HD)*(alibi - M).
    """
    jj = np.arange(128)[:, None]
    ii = np.arange(128)[None, :]
    rel = (jj - ii).astype(np.float64)          # (jj - ii)
    bias = np.empty((128, HPC, 17, 128), dtype=np.float32)
    for e in range(HPC):
        h = h0 + e
        slope = 2.0 ** (-8.0 * (h + 1) / H)
        bias[:, e, 0, :] = NEG
        for d in range(16):                      # d = it - jt >= 0
            v = SQHD * (slope * (rel - 128.0 * d) - M_OFF)
            tile_v = v.astype(np.float32)
            if d == 0:
                tile_v = np.where(jj > ii, NEG, tile_v)
            bias[:, e, d + 1, :] = tile_v
    return bias


_NC_CACHE = {}


def _get_program():
    if "nc" not in _NC_CACHE:
        _NC_CACHE["nc"] = build_program()
    return _NC_CACHE["nc"]


def make_in_maps(x, W_qkv, W_proj):
    x = np.asarray(x, dtype=np.float32)
    W_qkv = np.asarray(W_qkv, dtype=np.float32)
    W_proj = np.asarray(W_proj, dtype=np.float32)

    bf16 = mybir.dt.np(BF16)
    xT = np.ascontiguousarray(x.reshape(ROWS, DM).T).astype(bf16)   # [DM, ROWS]
    Wq, Wk, Wv = W_qkv[:, :DM], W_qkv[:, DM:2 * DM], W_qkv[:, 2 * DM:]
    cosT, sinT, ones128 = _host_tensors()

    in_maps = []
    for c in range(NCORES):
        h0 = HPC * c
        cols = np.r_[h0 * HD:(h0 + 1) * HD, (h0 + 1) * HD:(h0 + 2) * HD]
        in_maps.append({
            "xT": xT,
            "wq": np.ascontiguousarray(Wq[:, cols]).astype(bf16),
            "wk": np.ascontiguousarray(Wk[:, cols]).astype(bf16),
            "wv": np.ascontiguousarray(Wv[:, cols]).astype(bf16),
            "wp": np.ascontiguousarray(W_proj[cols, :]),
            "cosT": cosT,
            "sinT": sinT,
            "biasd": _bias_tiles(h0),
            "ones128": ones128,
        })
    return in_maps


def kernel(x, causal_mask, W_qkv, W_proj):
    del causal_mask  # always lower-triangular; causality is hardcoded
    nc = _get_program()
    in_maps = make_in_maps(x, W_qkv, W_proj)
    res = run_bass_kernel_spmd(nc, in_maps, core_ids=list(range(NCORES)))
    acc = np.zeros((ROWS, DM), dtype=np.float32)
    for c in range(NCORES):
        acc += res.results[c]["out"]
    return acc.reshape(B, T, DM)



# revision 23
# speedup vs baseline: 1.3698x; 1.3698x over previous
"""Trainium2 Bass kernel for CausalSelfAttention (RoPE + ALiBi + causal mask).

Sharding: 16 heads tensor-parallel across 8 NeuronCores (2 heads/core).
Per core:
  phase 1: qkv projection as 3-pass residual-compensated fp8e4m3 in
           DoubleRow perf mode (K=256/matmul, 0.5 cyc/row => ~4x f32r
           rate per pass; x8@W8 + xe8@W8 + x8@We8 recovers bf16-level
           accuracy). W streams are pre-scaled by 32 on the host to sit
           in e4m3's normal range; the PSUM->SBUF copy rescales by 1/32.
           RoPE applied via cross-partition DVE multiplies in bf16
           (all-2-byte operands hit the DVE 2x path). q,k,v kept bf16.
  phase 2: attention per (batch, head) in transposed layout
           S^T[j, i] = k^T.T @ q^T in bf16; ALiBi+mask added by DVE
           into bf16 SBUF; exp on ScalarE; row-sums via ones-matmul into
           a per-(b,icx) PSUM bank; y^T accumulated on TensorE;
           reciprocal broadcast on GpSimd. The normalized y is written
           as fp8 (y8) plus an fp8 residual (ye8) for phase 3.
  phase 3: out partial = y @ W_proj as 3-pass residual fp8 DoubleRow
           (the 2 heads form the K=256 pair), interleaved with phase 2
           per query chunk; output staged to bf16 and DMA'd per row
           block.
Host: sums the 8 bf16 partial outputs in f32.
"""

import math
from contextlib import ExitStack

import numpy as np

import concourse.bass as bass
import concourse.mybir as mybir
import concourse.tile as tile
from concourse import bacc
from concourse.bass_utils import run_bass_kernel_spmd

B, T, DM = 2, 2048, 2048
H, HD = 16, 128
ROWS = B * T                      # 4096
NCORES = 8
HPC = H // NCORES                 # 2 heads per core
ROPE_THETA = 10000.0
SQHD = math.sqrt(HD)
M_OFF = 18.0                      # softmax stability offset (applied in exp)
NEG = -1.0e30
XS = 16.0                         # host-side fp8 x scale
WS = 256.0                        # host-side fp8 weight scale
PS = XS * WS                      # product scale in the qkv / proj PSUM
YS = 16.0                         # device-side fp8 y scale
JCUT = 2                          # slot-0 ALiBi cutoff: drop jt < 4*icx-JCUT

TCH = 512                         # t-chunk width in phase 1
NCH = ROWS // TCH                 # 8
CT = DM // 128                    # 16 contraction tiles
NP = CT // 2                      # 8 DoubleRow k-tile pairs
NT = T // 128                     # 16 key/query tiles per batch
IC = 512                          # query chunk in phase 2
NIC = T // IC                     # 4

F32 = mybir.dt.float32
F32R = mybir.dt.float32r
BF16 = mybir.dt.bfloat16
F8 = mybir.dt.float8e4
MULT = mybir.AluOpType.mult
ADD = mybir.AluOpType.add
SUB = mybir.AluOpType.subtract
EXP = mybir.ActivationFunctionType.Exp
DR = mybir.MatmulPerfMode.DoubleRow


def build_program(phases="123", loop_n=1):
    nc = bacc.Bacc("TRN2", target_bir_lowering=False, debug=False,
                   num_devices=NCORES)
    x8 = nc.dram_tensor("x8", [DM, ROWS], F8, kind="ExternalInput").ap()
    xe8 = nc.dram_tensor("xe8", [DM, ROWS], F8, kind="ExternalInput").ap()
    wq8 = nc.dram_tensor("wq8", [DM, HPC * HD], F8, kind="ExternalInput").ap()
    wqe8 = nc.dram_tensor("wqe8", [DM, HPC * HD], F8, kind="ExternalInput").ap()
    wk8 = nc.dram_tensor("wk8", [DM, HPC * HD], F8, kind="ExternalInput").ap()
    wke8 = nc.dram_tensor("wke8", [DM, HPC * HD], F8, kind="ExternalInput").ap()
    wv8 = nc.dram_tensor("wv8", [DM, HPC * HD], F8, kind="ExternalInput").ap()
    wve8 = nc.dram_tensor("wve8", [DM, HPC * HD], F8, kind="ExternalInput").ap()
    wp8 = nc.dram_tensor("wp8", [HPC * HD, DM], F8, kind="ExternalInput").ap()
    wpe8 = nc.dram_tensor("wpe8", [HPC * HD, DM], F8, kind="ExternalInput").ap()
    cosT = nc.dram_tensor("cosT", [128, T], BF16, kind="ExternalInput").ap()
    sinT = nc.dram_tensor("sinT", [128, T], BF16, kind="ExternalInput").ap()
    biasd = nc.dram_tensor("biasd", [128, HPC, 17, 128], BF16,
                           kind="ExternalInput").ap()
    ones128 = nc.dram_tensor("ones128", [128, 1], BF16, kind="ExternalInput").ap()
    out = nc.dram_tensor("out", [ROWS, DM], BF16, kind="ExternalOutput").ap()

    x8_3 = x8.rearrange("(o p) t -> p o t", p=128)
    xe8_3 = xe8.rearrange("(o p) t -> p o t", p=128)

    def w3d(w):
        return w.rearrange("(o p) e -> p o e", p=128)

    do2 = "2" in phases
    do3 = "3" in phases

    with tile.TileContext(nc) as tc, ExitStack() as ctx:
        const = ctx.enter_context(tc.tile_pool(name="const", bufs=1))
        qkp = ctx.enter_context(tc.tile_pool(name="qk", bufs=1))
        # phase-2 pools live for the whole program: batch-0 attention runs
        # interleaved with the second half of the qkv phase
        wpool2 = ctx.enter_context(tc.tile_pool(name="wt", bufs=8))
        w32pool = ctx.enter_context(tc.tile_pool(name="w32", bufs=4))
        ypool = ctx.enter_context(tc.tile_pool(name="yb", bufs=2))
        yfpool = ctx.enter_context(tc.tile_pool(name="yf", bufs=2))
        smpool = ctx.enter_context(tc.tile_pool(name="sm", bufs=2))
        ostp = ctx.enter_context(tc.tile_pool(name="ost", bufs=3))
        pssc = ctx.enter_context(tc.tile_pool(name="pssc", bufs=2, space="PSUM"))
        psacc = ctx.enter_context(tc.tile_pool(name="psacc", bufs=2, space="PSUM"))
        psmisc = ctx.enter_context(tc.tile_pool(name="psm", bufs=1, space="PSUM"))

        q_sb = [qkp.tile([128, ROWS], BF16, tag=f"q{e}", name=f"q{e}")
                for e in range(HPC)]
        k_sb = [qkp.tile([128, ROWS], BF16, tag=f"k{e}", name=f"k{e}")
                for e in range(HPC)]
        v_keep = qkp.tile([128, B * NT, HPC * HD], BF16, tag="vk", name="vk")

        negm = const.tile([128, 1], F32, tag="negm")
        nc.gpsimd.memset(negm[:], -M_OFF)
        cos_sb = const.tile([128, T], BF16, tag="cos")
        sin_sb = const.tile([128, T], BF16, tag="sin")
        ones128_sb = const.tile([128, 1], BF16, tag="o128")
        bias_sb = const.tile([128, HPC, 17, 128], BF16, tag="bias")
        wp8_sb = const.tile([128, HPC, DM], F8, tag="wp8")
        wpe8_sb = const.tile([128, HPC, DM], F8, tag="wpe8")

        if loop_n > 1:
            # timing mode: run the whole body loop_n times on-device
            ctx.enter_context(tc.For_i(0, loop_n, 1))

        pending_proj = []
        pso_ref = [None]

        def emit_proj_tt(b, tt, y8_b, ye8_b):
            # projection of one 128-row block: 3-pass residual fp8
            # DoubleRow over the K=256 head pair
            tsl = slice(tt * 128, (tt + 1) * 128)
            o_big = ostp.tile([128, DM], BF16, tag="ost")
            for ec in range(DM // 512):
                esl = slice(ec * 512, (ec + 1) * 512)
                ps_out = pso_ref[0].tile([128, 512], F32, tag="pso")
                i = 0
                for ya, wa in ((y8_b, wp8_sb), (ye8_b, wp8_sb),
                               (y8_b, wpe8_sb)):
                    nc.tensor.matmul(ps_out[:],
                                     ya[:, :, tsl],
                                     wa[:, :, esl],
                                     start=(i == 0), stop=(i == 2),
                                     perf_mode=DR)
                    i += 1
                if (tt * 4 + ec) % 5 < 2:
                    nc.vector.tensor_scalar_mul(o_big[:, esl],
                                                ps_out[:], 1.0 / PS)
                else:
                    nc.scalar.mul(o_big[:, esl], ps_out[:], 1.0 / PS)
            r0 = b * T + tt * 128
            nc.sync.dma_start(out[r0:r0 + 128, :], o_big[:])

        def attn_icx(b, icx, y8_b, ye8_b, drain):
            """Generator: one (b, icx) attention chunk, yielding after each
            interleaved jt iteration so callers can weave other work in."""
            i0 = b * T + icx * IC
            jt_hi = (icx + 1) * (IC // 128)
            # slot-0 heads (host-assigned, slope >= 2^-4) decay below
            # e^-48 past ~385 keys: skip those tiles
            jt_lo = [max(0, 4 * icx - JCUT), 0]
            # both heads' row-sum rows live in one PSUM bank
            # (partitions 0 and 32 via matmul tile_position)
            ps_sp = psmisc.tile([33, IC], F32, tag="psm", name="psm")
            ps_sum = [ps_sp[0:1, :], ps_sp[32:33, :]]
            sum_pos = [(0, 0), (0, 32)]
            ps_y = [psacc.tile([128, IC], F32, tag="psy", name=f"psy{hi}")
                    for hi in range(HPC)]
            # the two heads' jt loops are interleaved so PE always has
            # the other head's matmuls during a tile's bias+exp chain
            for jt in range(jt_hi):
                # drain one pending projection row-block per jt iteration:
                # PE gets independent work while this jt's bias+exp chain
                # runs, without starving DVE/ACT
                if drain and pending_proj:
                    emit_proj_tt(*pending_proj.pop(0))
                    # drain faster near the end so the final flush is short
                    if b == 1 and icx == NIC - 1 and pending_proj:
                        emit_proj_tt(*pending_proj.pop(0))
                for hi in range(HPC):
                    if jt < jt_lo[hi]:
                        continue
                    # bf16 matmuls have no small-free-dim penalty, so
                    # clamp the causal offset exactly (128-aligned).
                    o = max(0, jt * 128 - icx * IC)
                    n = IC - o
                    ps_sc = pssc.tile([128, IC], F32, tag="pssc")
                    nc.tensor.matmul(
                        ps_sc[:, o:],
                        k_sb[hi][:, b * T + jt * 128: b * T + (jt + 1) * 128],
                        q_sb[hi][:, i0 + o:i0 + IC],
                        start=True, stop=True)
                    # bias slot s = (it - jt) + 1; slot 1 = diagonal
                    d0 = (icx * IC + o) // 128 - jt + 1
                    ps3 = ps_sc[:, o:].rearrange("p (a c) -> p a c", c=128)
                    # raw biased scores sit near s_raw+alibi with magnitudes
                    # up to ~1e3; stage them in f32r (bf16 ulp there would
                    # distort dominant weights), then exp emits compact bf16
                    # weights.
                    w32 = w32pool.tile([128, IC], F32R, tag="w32")
                    w3 = w32[:, o:].rearrange("p (a c) -> p a c", c=128)
                    nc.vector.scalar_tensor_tensor(
                        out=w3, in0=ps3, scalar=1.0,
                        in1=bias_sb[:, hi, d0:d0 + n // 128, :],
                        op0=MULT, op1=ADD)
                    w_t = wpool2.tile([128, IC], BF16, tag="wt")
                    nc.scalar.activation(w_t[:, o:], w32[:, o:], EXP,
                                         bias=negm[:], scale=1.0 / SQHD)
                    nc.tensor.matmul(ps_y[hi][:, o:],
                                     v_keep[:, b * NT + jt, hi * HD:(hi + 1) * HD],
                                     w_t[:, o:],
                                     start=(jt == jt_lo[hi]), stop=(jt == jt_hi - 1))
                    nc.tensor.matmul(ps_sum[hi][0:1, o:], ones128_sb[:], w_t[:, o:],
                                     start=(jt == jt_lo[hi]), stop=(jt == jt_hi - 1),
                                     tile_position=sum_pos[hi])
                yield
            bcas = []
            for hi in range(HPC):
                recip = smpool.tile([1, IC], F32R, tag=f"recip{hi}",
                                    name=f"recip{hi}")
                with nc.allow_low_precision(reason="f32r is 4-byte"):
                    nc.vector.reciprocal(recip[:], ps_sum[hi][:, :])
                # broadcast recip down 128 partitions on the (idle) gpsimd
                # engine; frees PE of the ones1 matmul and ACT of staging
                bca = smpool.tile([128, IC], F32R, tag=f"bca{hi}",
                                  name=f"bca{hi}")
                nc.gpsimd.partition_broadcast(bca[:], recip[0:1, :])
                bcas.append(bca)
            for hi in range(HPC):
                yf = yfpool.tile([128, IC], F32, tag="yf")
                nc.vector.tensor_mul(yf[:], ps_y[hi][:], bcas[hi][:])
                ics = slice(icx * IC, (icx + 1) * IC)
                # y staged in fp8 at 16x so the residual stays in e4m3's
                # normal range on hardware
                nc.scalar.mul(y8_b[:, hi, ics], yf[:], YS)
                nc.vector.scalar_tensor_tensor(
                    out=ye8_b[:, hi, ics], in0=yf[:], scalar=YS,
                    in1=y8_b[:, hi, ics], op0=MULT, op1=SUB)
            if do3:
                pending_proj.extend(
                    (b, tt, y8_b, ye8_b) for tt in
                    range(icx * (IC // 128), (icx + 1) * (IC // 128)))

        # ---------------- phase 1 (A+B): qkv + rope ----------------
        with tc.tile_pool(name="w1", bufs=1) as wpool, \
             tc.tile_pool(name="xt", bufs=2) as xpool, \
             tc.tile_pool(name="rope", bufs=3) as rpool, \
             tc.tile_pool(name="p1", bufs=2, space="PSUM") as ps1, \
             tc.tile_pool(name="pv", bufs=1, space="PSUM") as psv:
            # 6 weight tensors, each [128, CT, 256] fp8 (4KB/partition)
            wq8_sb = wpool.tile([128, CT, HPC * HD], F8, tag="wq8")
            wqe8_sb = wpool.tile([128, CT, HPC * HD], F8, tag="wqe8")
            wk8_sb = wpool.tile([128, CT, HPC * HD], F8, tag="wk8")
            wke8_sb = wpool.tile([128, CT, HPC * HD], F8, tag="wke8")
            wv8_sb = wpool.tile([128, CT, HPC * HD], F8, tag="wv8")
            wve8_sb = wpool.tile([128, CT, HPC * HD], F8, tag="wve8")

            def xchunk(tchunk):
                """DMA one chunk of x8 and xe8 (one DMA per stream: the SP
                sequencer serializes DMA issue at ~1.3us each)."""
                t0 = tchunk * TCH
                xa = xpool.tile([128, CT, TCH], F8, tag="x8")
                xb = xpool.tile([128, CT, TCH], F8, tag="xe8")
                nc.sync.dma_start(xa[:], x8_3[:, :, t0:t0 + TCH])
                nc.sync.dma_start(xb[:], xe8_3[:, :, t0:t0 + TCH])
                return xa, xb

            # DMA issue order is consumption order: chunk 0's first half
            # (split DMAs) and q/k weights lead; v weights + phase-2/3
            # constants trail.
            xa0 = xpool.tile([128, CT, TCH], F8, tag="x8", name="x8h")
            xb0 = xpool.tile([128, CT, TCH], F8, tag="xe8", name="xe8h")
            HW0 = TCH // 2
            nc.sync.dma_start(xa0[:, :, 0:HW0], x8_3[:, :, 0:HW0])
            nc.sync.dma_start(xb0[:, :, 0:HW0], xe8_3[:, :, 0:HW0])
            nc.sync.dma_start(wq8_sb[:], w3d(wq8))
            nc.sync.dma_start(wqe8_sb[:], w3d(wqe8))
            nc.sync.dma_start(wk8_sb[:], w3d(wk8))
            nc.sync.dma_start(wke8_sb[:], w3d(wke8))
            nc.sync.dma_start(cos_sb[:], cosT)
            nc.sync.dma_start(sin_sb[:], sinT)
            nc.sync.dma_start(xa0[:, :, HW0:], x8_3[:, :, HW0:TCH])
            nc.sync.dma_start(xb0[:, :, HW0:], xe8_3[:, :, HW0:TCH])
            nc.sync.dma_start(wv8_sb[:], w3d(wv8))
            nc.sync.dma_start(wve8_sb[:], w3d(wve8))
            xa1, xb1 = xchunk(1)
            nc.sync.dma_start(ones128_sb[:], ones128)
            nc.sync.dma_start(bias_sb[:], biasd)
            nc.sync.dma_start(wp8_sb[:], wp8.rearrange("(o p) e -> p o e", p=128))
            nc.sync.dma_start(wpe8_sb[:], wpe8.rearrange("(o p) e -> p o e", p=128))

            def chunk_groups(tchunk, xa, xb, c0=0, cw=TCH):
                """Closures for one chunk's column window [c0, c0+cw):
                q/k tile groups + v tile groups."""
                t0 = tchunk * TCH + c0
                cs = slice(t0 % T, t0 % T + cw)
                xsl = slice(c0, c0 + cw)
                groups = []

                def qk_group(dst, w8_sb, we8_sb, et):
                    ec = slice(et * HD, (et + 1) * HD)
                    ps_q = ps1.tile([128, cw], F32, tag="psq")
                    i = 0
                    for wt, xt in ((w8_sb, xa), (we8_sb, xa), (w8_sb, xb)):
                        for p in range(NP):
                            nc.tensor.matmul(
                                ps_q[:],
                                wt[:, 2 * p:2 * p + 2, ec],
                                xt[:, 2 * p:2 * p + 2, xsl],
                                start=(i == 0), stop=(i == 3 * NP - 1),
                                perf_mode=DR)
                            i += 1
                    qraw = rpool.tile([128, TCH], BF16, tag="qraw")
                    nc.scalar.mul(qraw[:, :cw], ps_q[:], 1.0 / PS)
                    # rotate-half via cross-partition DVE ops; sin_sb rows
                    # 64:128 hold -sin_h, rows 0:64 hold +sin_h. All
                    # operands bf16 => DVE 2x path.
                    tmp = rpool.tile([128, TCH], BF16, tag="tmp")
                    nc.vector.tensor_mul(tmp[0:64, :cw], qraw[64:128, :cw],
                                         sin_sb[64:128, cs])
                    nc.vector.tensor_mul(tmp[64:128, :cw], qraw[0:64, :cw],
                                         sin_sb[0:64, cs])
                    dcols = dst[et][:, t0:t0 + cw]
                    nc.vector.tensor_mul(dcols, qraw[:, :cw], cos_sb[:, cs])
                    nc.vector.tensor_add(dcols, dcols, tmp[:, :cw])

                def v_group(tt, ps_pair, slot):
                    ts = slice(tt * 128, (tt + 1) * 128)
                    i = 0
                    for wt, xt in ((wv8_sb, xa), (wve8_sb, xa), (wv8_sb, xb)):
                        for p in range(NP):
                            nc.tensor.matmul(
                                ps_pair[:, slot, :],
                                xt[:, 2 * p:2 * p + 2, ts],
                                wt[:, 2 * p:2 * p + 2, :],
                                start=(i == 0), stop=(i == 3 * NP - 1),
                                perf_mode=DR)
                            i += 1
                    nc.scalar.mul(v_keep[:, t0 // 128 + tt, :],
                                  ps_pair[:, slot, :], 1.0 / PS)

                for dst, w8_sb, we8_sb in ((q_sb, wq8_sb, wqe8_sb),
                                           (k_sb, wk8_sb, wke8_sb)):
                    for et in range(HPC):
                        groups.append((qk_group, (dst, w8_sb, we8_sb, et)))
                # two v outputs share one PSUM bank (packed halves)
                vstate = {}

                def v_wrap(tt):
                    if vstate.get("pair") is None:
                        vstate["pair"] = psv.tile([128, 2, HPC * HD], F32,
                                                  tag="psv", name="psv")
                        vstate["slot"] = 0
                    v_group(tt, vstate["pair"], vstate["slot"])
                    vstate["slot"] += 1
                    if vstate["slot"] == 2:
                        vstate["pair"] = None

                for tt in range(c0 // 128, (c0 + cw) // 128):
                    groups.append((v_wrap, (tt,)))
                return groups

            # phase A: chunks 0-3 (batch 0 rows), qkv only; chunk 0 is
            # processed as two half-windows so PE starts on the first half
            # while the second is still in flight
            for fn, args in chunk_groups(0, xa0, xb0, 0, TCH // 2):
                fn(*args)
            for fn, args in chunk_groups(0, xa0, xb0, TCH // 2, TCH // 2):
                fn(*args)
            for tchunk in range(1, 4):
                xa, xb = (xa1, xb1) if tchunk == 1 else xchunk(tchunk)
                for fn, args in chunk_groups(tchunk, xa, xb):
                    fn(*args)

            # phase B: chunks 4-7 (batch 1 rows) interleaved with batch-0
            # attention (PE-heavy qkv overlaps DVE/ACT-heavy attention);
            # batch-0 projections are deferred to phase C (no free PSUM)
            if do2:
                y8_b0 = ypool.tile([128, HPC, T], F8, tag="y8")
                ye8_b0 = ypool.tile([128, HPC, T], F8, tag="ye8")
            for tchunk in range(4, NCH):
                xa, xb = xchunk(tchunk)
                groups = chunk_groups(tchunk, xa, xb)
                if do2:
                    icx = tchunk - 4
                    feeder = attn_icx(0, icx, y8_b0, ye8_b0, drain=False)
                    jt_hi = (icx + 1) * (IC // 128)
                    done = 0
                    for g, (fn, args) in enumerate(groups):
                        fn(*args)
                        want = ((g + 1) * jt_hi + 7) // 8
                        while done < want and next(feeder, "end") != "end":
                            done += 1
                    for _ in feeder:
                        pass
                else:
                    for fn, args in groups:
                        fn(*args)

        # ---------------- phase C: batch-1 attention + all projections ----
        if do2:
            with tc.tile_pool(name="pso", bufs=3, space="PSUM") as pso:
                pso_ref[0] = pso
                y8_b1 = ypool.tile([128, HPC, T], F8, tag="y8")
                ye8_b1 = ypool.tile([128, HPC, T], F8, tag="ye8")
                for icx in range(NIC):
                    for _ in attn_icx(1, icx, y8_b1, ye8_b1, drain=do3):
                        pass
                for job in pending_proj:
                    emit_proj_tt(*job)

    nc.compile()
    return nc


def _host_tensors():
    """Core-independent constant inputs."""
    inv_freq = 1.0 / (ROPE_THETA ** (np.arange(0, HD, 2, dtype=np.float64) / HD))
    ang = np.arange(T, dtype=np.float64)[:, None] * inv_freq[None, :]   # [T, 64]
    bf16 = mybir.dt.np(BF16)
    cos_h = np.cos(ang).T.astype(np.float32)                            # [64, T]
    sin_h = np.sin(ang).T.astype(np.float32)
    cosT = np.concatenate([cos_h, cos_h], axis=0).astype(bf16)          # [128, T]
    # tmp[0:64] = q[64:128] * sinT[64:128] needs -sin there; tmp[64:128]
    # = q[0:64] * sinT[0:64] needs +sin (halves hold identical angles)
    sinT = np.concatenate([sin_h, -sin_h], axis=0).astype(bf16)

    ones128 = np.ones((128, 1), dtype=bf16)
    return cosT, sinT, ones128


def _bias_tiles(heads):
    """[128, HPC, 17, 128] additive pre-scale ALiBi bias, slot s = (it-jt)+1.

    Slot 0 (it < jt, fully masked) is all NEG; slot 1 (diagonal) has the
    upper triangle NEG; slots 2.. are pure sqrt(HD)*alibi. The -M_OFF
    stability offset is applied as the exp's constant bias so near-diagonal
    entries stay small enough for bf16.
    """
    jj = np.arange(128)[:, None]
    ii = np.arange(128)[None, :]
    rel = (jj - ii).astype(np.float64)          # (jj - ii)
    bias = np.empty((128, HPC, 17, 128), dtype=np.float32)
    for e, h in enumerate(heads):
        slope = 2.0 ** (-8.0 * (h + 1) / H)
        bias[:, e, 0, :] = NEG
        for d in range(16):                      # d = it - jt >= 0
            v = SQHD * slope * (rel - 128.0 * d)
            tile_v = v.astype(np.float32)
            if d == 0:
                tile_v = np.where(jj > ii, NEG, tile_v)
            bias[:, e, d + 1, :] = tile_v
    return bias.astype(mybir.dt.np(BF16))


_NC_CACHE = {}


def _get_program():
    if "nc" not in _NC_CACHE:
        _NC_CACHE["nc"] = build_program()
    return _NC_CACHE["nc"]


def _fp8_split(a, scale):
    """Return (fp8(a*scale), fp8(a*scale - fp8(a*scale))) as e4m3 arrays.

    The scale keeps both the main values and the residuals inside e4m3's
    normal range (subnormals may flush to zero on hardware).
    """
    f8 = mybir.dt.np(F8)
    hi = (a * scale).astype(f8)
    lo = (a * scale - hi.astype(np.float32)).astype(f8)
    return hi, lo


def core_heads(c):
    """Heads owned by core c: a high-slope head (slot 0, ALiBi cutoff
    applies) paired with a low-slope head (slot 1, full attention)."""
    return [c, c + NCORES]


def make_in_maps(x, W_qkv, W_proj):
    x = np.asarray(x, dtype=np.float32)
    W_qkv = np.asarray(W_qkv, dtype=np.float32)
    W_proj = np.asarray(W_proj, dtype=np.float32)

    xT = np.ascontiguousarray(x.reshape(ROWS, DM).T)                # [DM, ROWS]
    x8, xe8 = _fp8_split(xT, XS)
    Wq, Wk, Wv = W_qkv[:, :DM], W_qkv[:, DM:2 * DM], W_qkv[:, 2 * DM:]
    cosT, sinT, ones128 = _host_tensors()

    in_maps = []
    for c in range(NCORES):
        ha, hb = core_heads(c)
        cols = np.r_[ha * HD:(ha + 1) * HD, hb * HD:(hb + 1) * HD]
        wq8, wqe8 = _fp8_split(np.ascontiguousarray(Wq[:, cols]), WS)
        wk8, wke8 = _fp8_split(np.ascontiguousarray(Wk[:, cols]), WS)
        wv8, wve8 = _fp8_split(np.ascontiguousarray(Wv[:, cols]), WS)
        wp8, wpe8 = _fp8_split(np.ascontiguousarray(W_proj[cols, :]), WS)
        in_maps.append({
            "x8": x8,
            "xe8": xe8,
            "wq8": wq8, "wqe8": wqe8,
            "wk8": wk8, "wke8": wke8,
            "wv8": wv8, "wve8": wve8,
            "wp8": wp8, "wpe8": wpe8,
            "cosT": cosT,
            "sinT": sinT,
            "biasd": _bias_tiles(core_heads(c)),
            "ones128": ones128,
        })
    return in_maps


def kernel(x, causal_mask, W_qkv, W_proj):
    del causal_mask  # always lower-triangular; causality is hardcoded
    nc = _get_program()
    in_maps = make_in_maps(x, W_qkv, W_proj)
    res = run_bass_kernel_spmd(nc, in_maps, core_ids=list(range(NCORES)))
    acc = np.zeros((ROWS, DM), dtype=np.float32)
    for c in range(NCORES):
        acc += np.asarray(res.results[c]["out"], dtype=np.float32)
    return acc.reshape(B, T, DM)


# revision 30
# speedup vs baseline: 1.3965x; 1.0195x over previous
"""Trainium2 Bass kernel for CausalSelfAttention (RoPE + ALiBi + causal mask).

Sharding: 16 heads tensor-parallel across 8 NeuronCores; core c owns heads
{c, c+8} so every core pairs a high-slope ALiBi head (whose attention is
effectively local: key tiles >385 positions back contribute < e^-16 and are
skipped) with a low-slope full-attention head.

Per core, three overlapped regions:
  A: qkv projection for batch-0 rows as 3-pass residual-compensated
     fp8e4m3 in DoubleRow perf mode (K=256/matmul, 0.5 cyc/row => ~4x
     the f32r rate per pass; x8@W8 + xe8@W8 + x8@We8 recovers bf16-level
     accuracy). Host pre-scales x by 16 and W by 256 so both the fp8
     mains and the fp8 residuals sit in e4m3's normal range (hardware
     may flush subnormals); the PSUM->SBUF copy rescales by 1/4096.
     RoPE is applied by cross-partition DVE multiplies in bf16 (all
     2-byte operands hit the DVE 2x path). q,k,v are kept bf16 in SBUF.
  B: batch-1 qkv chunks (PE-heavy) interleaved with batch-0 attention
     (DVE/ACT-heavy): scores S^T[j,i] = k^T.T @ q^T in bf16; ALiBi+mask
     added by DVE from a bf16 table into an f32r staging tile (raw
     biased scores reach ~1e3 where bf16 ulp would distort the dominant
     weights); exp on ScalarE emits compact bf16 weights (the -18
     stability offset rides the exp's per-partition bias); y^T and
     row-sums accumulate on TensorE (both heads' row-sum rows share one
     PSUM bank via matmul tile_position); reciprocal on DVE, broadcast
     on GpSimd. Normalized y is staged as fp8 at 16x plus an fp8
     residual. Batch-0 projections are deferred (no free PSUM banks).
  C: batch-1 attention with projections woven in one row-block per key
     tile: out partial = y @ W_proj as 3-pass residual fp8 DoubleRow
     (the 2 heads form the K=256 pair), staged to bf16 and DMA'd per
     128-row block.
Host: sums the 8 bf16 partial outputs in f32.
"""

import math
from contextlib import ExitStack

import numpy as np

import concourse.bass as bass
import concourse.mybir as mybir
import concourse.tile as tile
from concourse import bacc
from concourse.bass_utils import run_bass_kernel_spmd

B, T, DM = 2, 2048, 2048
H, HD = 16, 128
ROWS = B * T                      # 4096
NCORES = 8
HPC = H // NCORES                 # 2 heads per core
ROPE_THETA = 10000.0
SQHD = math.sqrt(HD)
M_OFF = 18.0                      # softmax stability offset (applied in exp)
NEG = -1.0e30
XS = 16.0                         # host-side fp8 x scale
WS = 256.0                        # host-side fp8 weight scale
PS = XS * WS                      # product scale in the qkv / proj PSUM
YS = 16.0                         # device-side fp8 y scale
JCUT = 2                          # slot-0 ALiBi cutoff: drop jt < 4*icx-JCUT

TCH = 512                         # t-chunk width in phase 1
NCH = ROWS // TCH                 # 8
CT = DM // 128                    # 16 contraction tiles
NP = CT // 2                      # 8 DoubleRow k-tile pairs
NT = T // 128                     # 16 key/query tiles per batch
IC = 512                          # query chunk in phase 2
NIC = T // IC                     # 4

F32 = mybir.dt.float32
F32R = mybir.dt.float32r
BF16 = mybir.dt.bfloat16
F8 = mybir.dt.float8e4
MULT = mybir.AluOpType.mult
ADD = mybir.AluOpType.add
SUB = mybir.AluOpType.subtract
EXP = mybir.ActivationFunctionType.Exp
DR = mybir.MatmulPerfMode.DoubleRow


def build_program(phases="123", loop_n=1):
    nc = bacc.Bacc("TRN2", target_bir_lowering=False, debug=False,
                   num_devices=NCORES)
    x8 = nc.dram_tensor("x8", [DM, ROWS], F8, kind="ExternalInput").ap()
    xe8 = nc.dram_tensor("xe8", [DM, ROWS], F8, kind="ExternalInput").ap()
    wq8 = nc.dram_tensor("wq8", [DM, HPC * HD], F8, kind="ExternalInput").ap()
    wqe8 = nc.dram_tensor("wqe8", [DM, HPC * HD], F8, kind="ExternalInput").ap()
    wk8 = nc.dram_tensor("wk8", [DM, HPC * HD], F8, kind="ExternalInput").ap()
    wke8 = nc.dram_tensor("wke8", [DM, HPC * HD], F8, kind="ExternalInput").ap()
    wv8 = nc.dram_tensor("wv8", [DM, HPC * HD], F8, kind="ExternalInput").ap()
    wve8 = nc.dram_tensor("wve8", [DM, HPC * HD], F8, kind="ExternalInput").ap()
    wp8 = nc.dram_tensor("wp8", [HPC * HD, DM], F8, kind="ExternalInput").ap()
    wpe8 = nc.dram_tensor("wpe8", [HPC * HD, DM], F8, kind="ExternalInput").ap()
    cosT = nc.dram_tensor("cosT", [128, T], BF16, kind="ExternalInput").ap()
    sinT = nc.dram_tensor("sinT", [128, T], BF16, kind="ExternalInput").ap()
    biasd = nc.dram_tensor("biasd", [128, HPC, 17, 128], BF16,
                           kind="ExternalInput").ap()
    ones128 = nc.dram_tensor("ones128", [128, 1], BF16, kind="ExternalInput").ap()
    out = nc.dram_tensor("out", [ROWS, DM], BF16, kind="ExternalOutput").ap()

    x8_3 = x8.rearrange("(o p) t -> p o t", p=128)
    xe8_3 = xe8.rearrange("(o p) t -> p o t", p=128)

    def w3d(w):
        return w.rearrange("(o p) e -> p o e", p=128)

    do2 = "2" in phases
    do3 = "3" in phases

    with tile.TileContext(nc) as tc, ExitStack() as ctx:
        const = ctx.enter_context(tc.tile_pool(name="const", bufs=1))
        qkp = ctx.enter_context(tc.tile_pool(name="qk", bufs=1))
        # phase-2 pools live for the whole program: batch-0 attention runs
        # interleaved with the second half of the qkv phase
        wpool2 = ctx.enter_context(tc.tile_pool(name="wt", bufs=12))
        w32pool = ctx.enter_context(tc.tile_pool(name="w32", bufs=6))
        ypool = ctx.enter_context(tc.tile_pool(name="yb", bufs=2))
        yfpool = ctx.enter_context(tc.tile_pool(name="yf", bufs=2))
        smpool = ctx.enter_context(tc.tile_pool(name="sm", bufs=2))
        ostp = ctx.enter_context(tc.tile_pool(name="ost", bufs=3))
        pssc = ctx.enter_context(tc.tile_pool(name="pssc", bufs=2, space="PSUM"))
        psacc = ctx.enter_context(tc.tile_pool(name="psacc", bufs=2, space="PSUM"))
        psmisc = ctx.enter_context(tc.tile_pool(name="psm", bufs=1, space="PSUM"))

        q_sb = [qkp.tile([128, ROWS], BF16, tag=f"q{e}", name=f"q{e}")
                for e in range(HPC)]
        k_sb = [qkp.tile([128, ROWS], BF16, tag=f"k{e}", name=f"k{e}")
                for e in range(HPC)]
        v_keep = qkp.tile([128, B * NT, HPC * HD], BF16, tag="vk", name="vk")

        negm = const.tile([128, 1], F32, tag="negm")
        nc.gpsimd.memset(negm[:], -M_OFF)
        cos_sb = const.tile([128, T], BF16, tag="cos")
        sin_sb = const.tile([128, T], BF16, tag="sin")
        ones128_sb = const.tile([128, 1], BF16, tag="o128")
        bias_sb = const.tile([128, HPC, 17, 128], BF16, tag="bias")
        wp8_sb = const.tile([128, HPC, DM], F8, tag="wp8")
        wpe8_sb = const.tile([128, HPC, DM], F8, tag="wpe8")

        if loop_n > 1:
            # timing mode: run the whole body loop_n times on-device
            ctx.enter_context(tc.For_i(0, loop_n, 1))

        pending_proj = []
        pso_ref = [None]

        def emit_proj_tt(b, tt, y8_b, ye8_b):
            # projection of one 128-row block: 3-pass residual fp8
            # DoubleRow over the K=256 head pair
            tsl = slice(tt * 128, (tt + 1) * 128)
            o_big = ostp.tile([128, DM], BF16, tag="ost")
            for ec in range(DM // 512):
                esl = slice(ec * 512, (ec + 1) * 512)
                ps_out = pso_ref[0].tile([128, 512], F32, tag="pso")
                i = 0
                for ya, wa in ((y8_b, wp8_sb), (ye8_b, wp8_sb),
                               (y8_b, wpe8_sb)):
                    nc.tensor.matmul(ps_out[:],
                                     ya[:, :, tsl],
                                     wa[:, :, esl],
                                     start=(i == 0), stop=(i == 2),
                                     perf_mode=DR)
                    i += 1
                if (tt * 4 + ec) % 5 < 2:
                    nc.vector.tensor_scalar_mul(o_big[:, esl],
                                                ps_out[:], 1.0 / PS)
                else:
                    nc.scalar.mul(o_big[:, esl], ps_out[:], 1.0 / PS)
            r0 = b * T + tt * 128
            nc.sync.dma_start(out[r0:r0 + 128, :], o_big[:])

        def attn_icx(b, icx, y8_b, ye8_b, drain):
            """Generator: one (b, icx) attention chunk, yielding after each
            interleaved jt iteration so callers can weave other work in."""
            i0 = b * T + icx * IC
            jt_hi = (icx + 1) * (IC // 128)
            # slot-0 heads (host-assigned, slope >= 2^-4) decay below
            # e^-48 past ~385 keys: skip those tiles
            jt_lo = [max(0, 4 * icx - JCUT), 0]
            # both heads' row-sum rows live in one PSUM bank
            # (partitions 0 and 32 via matmul tile_position)
            ps_sp = psmisc.tile([33, IC], F32, tag="psm", name="psm")
            ps_sum = [ps_sp[0:1, :], ps_sp[32:33, :]]
            sum_pos = [(0, 0), (0, 32)]
            ps_y = [psacc.tile([128, IC], F32, tag="psy", name=f"psy{hi}")
                    for hi in range(HPC)]
            # the two heads' jt loops are interleaved so PE always has
            # the other head's matmuls during a tile's bias+exp chain
            for jt in range(jt_hi):
                # drain one pending projection row-block per jt iteration:
                # PE gets independent work while this jt's bias+exp chain
                # runs, without starving DVE/ACT
                if drain and pending_proj:
                    emit_proj_tt(*pending_proj.pop(0))
                    # drain faster near the end so the final flush is short
                    if b == 1 and icx == NIC - 1 and pending_proj:
                        emit_proj_tt(*pending_proj.pop(0))
                for hi in range(HPC):
                    if jt < jt_lo[hi]:
                        continue
                    # bf16 matmuls have no small-free-dim penalty, so
                    # clamp the causal offset exactly (128-aligned).
                    o = max(0, jt * 128 - icx * IC)
                    n = IC - o
                    ps_sc = pssc.tile([128, IC], F32, tag="pssc")
                    nc.tensor.matmul(
                        ps_sc[:, o:],
                        k_sb[hi][:, b * T + jt * 128: b * T + (jt + 1) * 128],
                        q_sb[hi][:, i0 + o:i0 + IC],
                        start=True, stop=True)
                    # bias slot s = (it - jt) + 1; slot 1 = diagonal
                    d0 = (icx * IC + o) // 128 - jt + 1
                    ps3 = ps_sc[:, o:].rearrange("p (a c) -> p a c", c=128)
                    # raw biased scores sit near s_raw+alibi with magnitudes
                    # up to ~1e3; stage them in f32r (bf16 ulp there would
                    # distort dominant weights), then exp emits compact bf16
                    # weights.
                    w32 = w32pool.tile([128, IC], F32R, tag="w32")
                    w3 = w32[:, o:].rearrange("p (a c) -> p a c", c=128)
                    nc.vector.scalar_tensor_tensor(
                        out=w3, in0=ps3, scalar=1.0,
                        in1=bias_sb[:, hi, d0:d0 + n // 128, :],
                        op0=MULT, op1=ADD)
                    w_t = wpool2.tile([128, IC], BF16, tag="wt")
                    nc.scalar.activation(w_t[:, o:], w32[:, o:], EXP,
                                         bias=negm[:], scale=1.0 / SQHD)
                    nc.tensor.matmul(ps_y[hi][:, o:],
                                     v_keep[:, b * NT + jt, hi * HD:(hi + 1) * HD],
                                     w_t[:, o:],
                                     start=(jt == jt_lo[hi]), stop=(jt == jt_hi - 1))
                    nc.tensor.matmul(ps_sum[hi][0:1, o:], ones128_sb[:], w_t[:, o:],
                                     start=(jt == jt_lo[hi]), stop=(jt == jt_hi - 1),
                                     tile_position=sum_pos[hi])
                yield
            bcas = []
            for hi in range(HPC):
                recip = smpool.tile([1, IC], F32R, tag=f"recip{hi}",
                                    name=f"recip{hi}")
                with nc.allow_low_precision(reason="f32r is 4-byte"):
                    nc.vector.reciprocal(recip[:], ps_sum[hi][:, :])
                # broadcast recip down 128 partitions on the (idle) gpsimd
                # engine; frees PE of the ones1 matmul and ACT of staging
                bca = smpool.tile([128, IC], F32R, tag=f"bca{hi}",
                                  name=f"bca{hi}")
                nc.gpsimd.partition_broadcast(bca[:], recip[0:1, :])
                bcas.append(bca)
            for hi in range(HPC):
                yf = yfpool.tile([128, IC], F32, tag="yf")
                nc.vector.tensor_mul(yf[:], ps_y[hi][:], bcas[hi][:])
                ics = slice(icx * IC, (icx + 1) * IC)
                # y staged in fp8 at 16x so the residual stays in e4m3's
                # normal range on hardware
                nc.scalar.mul(y8_b[:, hi, ics], yf[:], YS)
                nc.vector.scalar_tensor_tensor(
                    out=ye8_b[:, hi, ics], in0=yf[:], scalar=YS,
                    in1=y8_b[:, hi, ics], op0=MULT, op1=SUB)
            if do3:
                pending_proj.extend(
                    (b, tt, y8_b, ye8_b) for tt in
                    range(icx * (IC // 128), (icx + 1) * (IC // 128)))

        # ---------------- phase 1 (A+B): qkv + rope ----------------
        with tc.tile_pool(name="w1", bufs=1) as wpool, \
             tc.tile_pool(name="xt", bufs=2) as xpool, \
             tc.tile_pool(name="rope", bufs=3) as rpool, \
             tc.tile_pool(name="p1", bufs=2, space="PSUM") as ps1, \
             tc.tile_pool(name="pv", bufs=1, space="PSUM") as psv:
            # 6 weight tensors, each [128, CT, 256] fp8 (4KB/partition)
            wq8_sb = wpool.tile([128, CT, HPC * HD], F8, tag="wq8")
            wqe8_sb = wpool.tile([128, CT, HPC * HD], F8, tag="wqe8")
            wk8_sb = wpool.tile([128, CT, HPC * HD], F8, tag="wk8")
            wke8_sb = wpool.tile([128, CT, HPC * HD], F8, tag="wke8")
            wv8_sb = wpool.tile([128, CT, HPC * HD], F8, tag="wv8")
            wve8_sb = wpool.tile([128, CT, HPC * HD], F8, tag="wve8")

            def xchunk(tchunk):
                """DMA one chunk of x8 and xe8 (one DMA per stream: the SP
                sequencer serializes DMA issue at ~1.3us each)."""
                t0 = tchunk * TCH
                xa = xpool.tile([128, CT, TCH], F8, tag="x8")
                xb = xpool.tile([128, CT, TCH], F8, tag="xe8")
                # split issue across the two HWDGE queues (SP + ACT): the
                # sequencers serialize DMA issue at ~1.3us each
                nc.sync.dma_start(xa[:], x8_3[:, :, t0:t0 + TCH])
                nc.scalar.dma_start(xb[:], xe8_3[:, :, t0:t0 + TCH])
                return xa, xb

            # DMA issue order is consumption order: chunk 0's q/k weights and
            # x lead; v weights + phase-2/3 constants trail.
            xa0, xb0 = xchunk(0)
            nc.scalar.dma_start(wq8_sb[:], w3d(wq8))
            nc.scalar.dma_start(wqe8_sb[:], w3d(wqe8))
            nc.sync.dma_start(wk8_sb[:], w3d(wk8))
            nc.sync.dma_start(wke8_sb[:], w3d(wke8))
            nc.sync.dma_start(wv8_sb[:], w3d(wv8))
            nc.sync.dma_start(wve8_sb[:], w3d(wve8))
            nc.sync.dma_start(cos_sb[:], cosT)
            nc.sync.dma_start(sin_sb[:], sinT)
            xa1, xb1 = xchunk(1)
            nc.sync.dma_start(ones128_sb[:], ones128)
            nc.sync.dma_start(bias_sb[:], biasd)
            nc.sync.dma_start(wp8_sb[:], wp8.rearrange("(o p) e -> p o e", p=128))
            nc.sync.dma_start(wpe8_sb[:], wpe8.rearrange("(o p) e -> p o e", p=128))

            def chunk_groups(tchunk, xa, xb, c0=0, cw=TCH):
                """Closures for one chunk's column window [c0, c0+cw):
                q/k tile groups + v tile groups."""
                t0 = tchunk * TCH + c0
                cs = slice(t0 % T, t0 % T + cw)
                xsl = slice(c0, c0 + cw)
                groups = []

                def qk_group(dst, w8_sb, we8_sb, et):
                    ec = slice(et * HD, (et + 1) * HD)
                    ps_q = ps1.tile([128, cw], F32, tag="psq")
                    i = 0
                    for wt, xt in ((w8_sb, xa), (we8_sb, xa), (w8_sb, xb)):
                        for p in range(NP):
                            nc.tensor.matmul(
                                ps_q[:],
                                wt[:, 2 * p:2 * p + 2, ec],
                                xt[:, 2 * p:2 * p + 2, xsl],
                                start=(i == 0), stop=(i == 3 * NP - 1),
                                perf_mode=DR)
                            i += 1
                    qraw = rpool.tile([128, TCH], BF16, tag="qraw")
                    nc.scalar.mul(qraw[:, :cw], ps_q[:], 1.0 / PS)
                    # rotate-half via cross-partition DVE ops; sin_sb rows
                    # 64:128 hold -sin_h, rows 0:64 hold +sin_h. All
                    # operands bf16 => DVE 2x path.
                    tmp = rpool.tile([128, TCH], BF16, tag="tmp")
                    nc.vector.tensor_mul(tmp[0:64, :cw], qraw[64:128, :cw],
                                         sin_sb[64:128, cs])
                    nc.vector.tensor_mul(tmp[64:128, :cw], qraw[0:64, :cw],
                                         sin_sb[0:64, cs])
                    dcols = dst[et][:, t0:t0 + cw]
                    nc.vector.tensor_mul(dcols, qraw[:, :cw], cos_sb[:, cs])
                    nc.vector.tensor_add(dcols, dcols, tmp[:, :cw])

                def v_group(tt, ps_pair, slot):
                    ts = slice(tt * 128, (tt + 1) * 128)
                    i = 0
                    for wt, xt in ((wv8_sb, xa), (wve8_sb, xa), (wv8_sb, xb)):
                        for p in range(NP):
                            nc.tensor.matmul(
                                ps_pair[:, slot, :],
                                xt[:, 2 * p:2 * p + 2, ts],
                                wt[:, 2 * p:2 * p + 2, :],
                                start=(i == 0), stop=(i == 3 * NP - 1),
                                perf_mode=DR)
                            i += 1
                    nc.scalar.mul(v_keep[:, t0 // 128 + tt, :],
                                  ps_pair[:, slot, :], 1.0 / PS)

                for dst, w8_sb, we8_sb in ((q_sb, wq8_sb, wqe8_sb),
                                           (k_sb, wk8_sb, wke8_sb)):
                    for et in range(HPC):
                        groups.append((qk_group, (dst, w8_sb, we8_sb, et)))
                # two v outputs share one PSUM bank (packed halves)
                vstate = {}

                def v_wrap(tt):
                    if vstate.get("pair") is None:
                        vstate["pair"] = psv.tile([128, 2, HPC * HD], F32,
                                                  tag="psv", name="psv")
                        vstate["slot"] = 0
                    v_group(tt, vstate["pair"], vstate["slot"])
                    vstate["slot"] += 1
                    if vstate["slot"] == 2:
                        vstate["pair"] = None

                for tt in range(c0 // 128, (c0 + cw) // 128):
                    groups.append((v_wrap, (tt,)))
                return groups

            # phase A: chunks 0-3 (batch 0 rows), qkv only
            for tchunk in range(4):
                if tchunk == 0:
                    xa, xb = xa0, xb0
                elif tchunk == 1:
                    xa, xb = xa1, xb1
                else:
                    xa, xb = xchunk(tchunk)
                for fn, args in chunk_groups(tchunk, xa, xb):
                    fn(*args)

            # phase B: chunks 4-7 (batch 1 rows) interleaved with batch-0
            # attention (PE-heavy qkv overlaps DVE/ACT-heavy attention);
            # batch-0 projections are deferred to phase C (no free PSUM)
            if do2:
                y8_b0 = ypool.tile([128, HPC, T], F8, tag="y8")
                ye8_b0 = ypool.tile([128, HPC, T], F8, tag="ye8")
            for tchunk in range(4, NCH):
                xa, xb = xchunk(tchunk)
                groups = chunk_groups(tchunk, xa, xb)
                if do2:
                    icx = tchunk - 4
                    feeder = attn_icx(0, icx, y8_b0, ye8_b0, drain=False)
                    jt_hi = (icx + 1) * (IC // 128)
                    done = 0
                    for g, (fn, args) in enumerate(groups):
                        fn(*args)
                        want = ((g + 1) * jt_hi + 7) // 8
                        while done < want and next(feeder, "end") != "end":
                            done += 1
                    for _ in feeder:
                        pass
                else:
                    for fn, args in groups:
                        fn(*args)

        # ---------------- phase C: batch-1 attention + all projections ----
        if do2:
            with tc.tile_pool(name="pso", bufs=3, space="PSUM") as pso:
                pso_ref[0] = pso
                y8_b1 = ypool.tile([128, HPC, T], F8, tag="y8")
                ye8_b1 = ypool.tile([128, HPC, T], F8, tag="ye8")
                for icx in range(NIC):
                    for _ in attn_icx(1, icx, y8_b1, ye8_b1, drain=do3):
                        pass
                for job in pending_proj:
                    emit_proj_tt(*job)

    nc.compile()
    return nc


def _host_tensors():
    """Core-independent constant inputs."""
    inv_freq = 1.0 / (ROPE_THETA ** (np.arange(0, HD, 2, dtype=np.float64) / HD))
    ang = np.arange(T, dtype=np.float64)[:, None] * inv_freq[None, :]   # [T, 64]
    bf16 = mybir.dt.np(BF16)
    cos_h = np.cos(ang).T.astype(np.float32)                            # [64, T]
    sin_h = np.sin(ang).T.astype(np.float32)
    cosT = np.concatenate([cos_h, cos_h], axis=0).astype(bf16)          # [128, T]
    # tmp[0:64] = q[64:128] * sinT[64:128] needs -sin there; tmp[64:128]
    # = q[0:64] * sinT[0:64] needs +sin (halves hold identical angles)
    sinT = np.concatenate([sin_h, -sin_h], axis=0).astype(bf16)

    ones128 = np.ones((128, 1), dtype=bf16)
    return cosT, sinT, ones128


def _bias_tiles(heads):
    """[128, HPC, 17, 128] additive pre-scale ALiBi bias, slot s = (it-jt)+1.

    Slot 0 (it < jt, fully masked) is all NEG; slot 1 (diagonal) has the
    upper triangle NEG; slots 2.. are pure sqrt(HD)*alibi. The -M_OFF
    stability offset is applied as the exp's constant bias so near-diagonal
    entries stay small enough for bf16.
    """
    jj = np.arange(128)[:, None]
    ii = np.arange(128)[None, :]
    rel = (jj - ii).astype(np.float64)          # (jj - ii)
    bias = np.empty((128, HPC, 17, 128), dtype=np.float32)
    for e, h in enumerate(heads):
        slope = 2.0 ** (-8.0 * (h + 1) / H)
        bias[:, e, 0, :] = NEG
        for d in range(16):                      # d = it - jt >= 0
            v = SQHD * slope * (rel - 128.0 * d)
            tile_v = v.astype(np.float32)
            if d == 0:
                tile_v = np.where(jj > ii, NEG, tile_v)
            bias[:, e, d + 1, :] = tile_v
    return bias.astype(mybir.dt.np(BF16))


_NC_CACHE = {}


def _get_program():
    if "nc" not in _NC_CACHE:
        _NC_CACHE["nc"] = build_program()
    return _NC_CACHE["nc"]


def _fp8_split(a, scale):
    """Return (fp8(a*scale), fp8(a*scale - fp8(a*scale))) as e4m3 arrays.

    The scale keeps both the main values and the residuals inside e4m3's
    normal range (subnormals may flush to zero on hardware).
    """
    f8 = mybir.dt.np(F8)
    hi = (a * scale).astype(f8)
    lo = (a * scale - hi.astype(np.float32)).astype(f8)
    return hi, lo


def core_heads(c):
    """Heads owned by core c: a high-slope head (slot 0, ALiBi cutoff
    applies) paired with a low-slope head (slot 1, full attention)."""
    return [c, c + NCORES]


def make_in_maps(x, W_qkv, W_proj):
    x = np.asarray(x, dtype=np.float32)
    W_qkv = np.asarray(W_qkv, dtype=np.float32)
    W_proj = np.asarray(W_proj, dtype=np.float32)

    xT = np.ascontiguousarray(x.reshape(ROWS, DM).T)                # [DM, ROWS]
    x8, xe8 = _fp8_split(xT, XS)
    Wq, Wk, Wv = W_qkv[:, :DM], W_qkv[:, DM:2 * DM], W_qkv[:, 2 * DM:]
    cosT, sinT, ones128 = _host_tensors()

    in_maps = []
    for c in range(NCORES):
        ha, hb = core_heads(c)
        cols = np.r_[ha * HD:(ha + 1) * HD, hb * HD:(hb + 1) * HD]
        wq8, wqe8 = _fp8_split(np.ascontiguousarray(Wq[:, cols]), WS)
        wk8, wke8 = _fp8_split(np.ascontiguousarray(Wk[:, cols]), WS)
        wv8, wve8 = _fp8_split(np.ascontiguousarray(Wv[:, cols]), WS)
        wp8, wpe8 = _fp8_split(np.ascontiguousarray(W_proj[cols, :]), WS)
        in_maps.append({
            "x8": x8,
            "xe8": xe8,
            "wq8": wq8, "wqe8": wqe8,
            "wk8": wk8, "wke8": wke8,
            "wv8": wv8, "wve8": wve8,
            "wp8": wp8, "wpe8": wpe8,
            "cosT": cosT,
            "sinT": sinT,
            "biasd": _bias_tiles(core_heads(c)),
            "ones128": ones128,
        })
    return in_maps


def kernel(x, causal_mask, W_qkv, W_proj):
    del causal_mask  # always lower-triangular; causality is hardcoded
    nc = _get_program()
    in_maps = make_in_maps(x, W_qkv, W_proj)
    res = run_bass_kernel_spmd(nc, in_maps, core_ids=list(range(NCORES)))
    acc = np.zeros((ROWS, DM), dtype=np.float32)
    for c in range(NCORES):
        acc += np.asarray(res.results[c]["out"], dtype=np.float32)
    return acc.reshape(B, T, DM)


# revision 37
# speedup vs baseline: 1.4260x; 1.0211x over previous
"""Trainium2 Bass kernel for CausalSelfAttention (RoPE + ALiBi + causal mask).

Sharding: 16 heads tensor-parallel across 8 NeuronCores; core c owns heads
{c, c+8} so every core pairs a high-slope ALiBi head (whose attention is
effectively local: key tiles >385 positions back contribute < e^-16 and are
skipped) with a low-slope full-attention head.

Per core, three overlapped regions:
  A: qkv projection for batch-0 rows as 3-pass residual-compensated
     fp8e4m3 in DoubleRow perf mode (K=256/matmul, 0.5 cyc/row => ~4x
     the f32r rate per pass; x8@W8 + xe8@W8 + x8@We8 recovers bf16-level
     accuracy). Host pre-scales x by 16 and W by 256 so both the fp8
     mains and the fp8 residuals sit in e4m3's normal range (hardware
     may flush subnormals); the PSUM->SBUF copy rescales by 1/4096.
     RoPE is applied by cross-partition DVE multiplies in bf16 (all
     2-byte operands hit the DVE 2x path). q,k,v are kept bf16 in SBUF.
  B: batch-1 qkv chunks (PE-heavy) interleaved with batch-0 attention
     (DVE/ACT-heavy): scores S^T[j,i] = k^T.T @ q^T in bf16; ALiBi+mask
     added by DVE from a bf16 table into an f32r staging tile (raw
     biased scores reach ~1e3 where bf16 ulp would distort the dominant
     weights); exp on ScalarE emits compact bf16 weights (the -18
     stability offset rides the exp's per-partition bias); y^T and
     row-sums accumulate on TensorE (both heads' row-sum rows share one
     PSUM bank via matmul tile_position); reciprocal on DVE, broadcast
     on GpSimd. Normalized y is staged as fp8 at 16x plus an fp8
     residual. Batch-0 projections are deferred (no free PSUM banks).
  C: batch-1 attention with projections woven in one row-block per key
     tile: out partial = y @ W_proj as 3-pass residual fp8 DoubleRow
     (the 2 heads form the K=256 pair), staged to bf16 and DMA'd per
     128-row block.
Host: sums the 8 bf16 partial outputs in f32.
"""

import math
from contextlib import ExitStack

import numpy as np

import concourse.bass as bass
import concourse.mybir as mybir
import concourse.tile as tile
from concourse import bacc
from concourse.bass_utils import run_bass_kernel_spmd

B, T, DM = 2, 2048, 2048
H, HD = 16, 128
ROWS = B * T                      # 4096
NCORES = 8
HPC = H // NCORES                 # 2 heads per core
ROPE_THETA = 10000.0
SQHD = math.sqrt(HD)
M_OFF = 18.0                      # softmax stability offset (applied in exp)
NEG = -1.0e30
XS = 16.0                         # host-side fp8 x scale
WS = 256.0                        # host-side fp8 weight scale
PS = XS * WS                      # product scale in the qkv / proj PSUM
YS = 16.0                         # device-side fp8 y scale
JCUT = 2                          # slot-0 ALiBi cutoff: drop jt < 4*icx-JCUT

TCH = 512                         # t-chunk width in phase 1
NCH = ROWS // TCH                 # 8
CT = DM // 128                    # 16 contraction tiles
NP = CT // 2                      # 8 DoubleRow k-tile pairs
NT = T // 128                     # 16 key/query tiles per batch
IC = 512                          # query chunk in phase 2
NIC = T // IC                     # 4

F32 = mybir.dt.float32
F32R = mybir.dt.float32r
BF16 = mybir.dt.bfloat16
F8 = mybir.dt.float8e4
MULT = mybir.AluOpType.mult
ADD = mybir.AluOpType.add
SUB = mybir.AluOpType.subtract
EXP = mybir.ActivationFunctionType.Exp
DR = mybir.MatmulPerfMode.DoubleRow


def build_program(phases="123", loop_n=1):
    nc = bacc.Bacc("TRN2", target_bir_lowering=False, debug=False,
                   num_devices=NCORES)
    # x and weight streams are stored pre-tiled in their SBUF layouts so
    # each DMA is 128 long contiguous per-partition descriptors (the
    # natural [DM, ROWS] layout would emit 2048 tiny descriptors per DMA)
    x8 = nc.dram_tensor("x8", [NCH, 128, CT, TCH], F8, kind="ExternalInput").ap()
    xe8 = nc.dram_tensor("xe8", [NCH, 128, CT, TCH], F8, kind="ExternalInput").ap()
    wq8 = nc.dram_tensor("wq8", [128, CT, HPC * HD], F8, kind="ExternalInput").ap()
    wqe8 = nc.dram_tensor("wqe8", [128, CT, HPC * HD], F8, kind="ExternalInput").ap()
    wk8 = nc.dram_tensor("wk8", [128, CT, HPC * HD], F8, kind="ExternalInput").ap()
    wke8 = nc.dram_tensor("wke8", [128, CT, HPC * HD], F8, kind="ExternalInput").ap()
    wv8 = nc.dram_tensor("wv8", [128, CT, HPC * HD], F8, kind="ExternalInput").ap()
    wve8 = nc.dram_tensor("wve8", [128, CT, HPC * HD], F8, kind="ExternalInput").ap()
    wp8 = nc.dram_tensor("wp8", [128, HPC, DM], F8, kind="ExternalInput").ap()
    wpe8 = nc.dram_tensor("wpe8", [128, HPC, DM], F8, kind="ExternalInput").ap()
    cosT = nc.dram_tensor("cosT", [128, T], BF16, kind="ExternalInput").ap()
    sinT = nc.dram_tensor("sinT", [128, T], BF16, kind="ExternalInput").ap()
    biasd = nc.dram_tensor("biasd", [128, HPC, 17, 128], BF16,
                           kind="ExternalInput").ap()
    ones128 = nc.dram_tensor("ones128", [128, 1], BF16, kind="ExternalInput").ap()
    out = nc.dram_tensor("out", [ROWS, DM], BF16, kind="ExternalOutput").ap()

    do2 = "2" in phases
    do3 = "3" in phases

    with tile.TileContext(nc) as tc, ExitStack() as ctx:
        const = ctx.enter_context(tc.tile_pool(name="const", bufs=1))
        qkp = ctx.enter_context(tc.tile_pool(name="qk", bufs=1))
        # phase-2 pools live for the whole program: batch-0 attention runs
        # interleaved with the second half of the qkv phase
        wpool2 = ctx.enter_context(tc.tile_pool(name="wt", bufs=12))
        w32pool = ctx.enter_context(tc.tile_pool(name="w32", bufs=6))
        ypool = ctx.enter_context(tc.tile_pool(name="yb", bufs=2))
        yfpool = ctx.enter_context(tc.tile_pool(name="yf", bufs=2))
        smpool = ctx.enter_context(tc.tile_pool(name="sm", bufs=2))
        ostp = ctx.enter_context(tc.tile_pool(name="ost", bufs=3))
        pssc = ctx.enter_context(tc.tile_pool(name="pssc", bufs=2, space="PSUM"))
        psacc = ctx.enter_context(tc.tile_pool(name="psacc", bufs=2, space="PSUM"))
        psmisc = ctx.enter_context(tc.tile_pool(name="psm", bufs=1, space="PSUM"))

        q_sb = [qkp.tile([128, ROWS], BF16, tag=f"q{e}", name=f"q{e}")
                for e in range(HPC)]
        k_sb = [qkp.tile([128, ROWS], BF16, tag=f"k{e}", name=f"k{e}")
                for e in range(HPC)]
        v_keep = qkp.tile([128, B * NT, HPC * HD], BF16, tag="vk", name="vk")

        negm = const.tile([128, 1], F32, tag="negm")
        nc.gpsimd.memset(negm[:], -M_OFF)
        cos_sb = const.tile([128, T], BF16, tag="cos")
        sin_sb = const.tile([128, T], BF16, tag="sin")
        ones128_sb = const.tile([128, 1], BF16, tag="o128")
        bias_sb = const.tile([128, HPC, 17, 128], BF16, tag="bias")
        wp8_sb = const.tile([128, HPC, DM], F8, tag="wp8")
        wpe8_sb = const.tile([128, HPC, DM], F8, tag="wpe8")

        if loop_n > 1:
            # timing mode: run the whole body loop_n times on-device
            ctx.enter_context(tc.For_i(0, loop_n, 1))

        pending_proj = []
        pso_ref = [None]

        def emit_proj_tt(b, tt, y8_b, ye8_b):
            # projection of one 128-row block: 3-pass residual fp8
            # DoubleRow over the K=256 head pair
            tsl = slice(tt * 128, (tt + 1) * 128)
            o_big = ostp.tile([128, DM], BF16, tag="ost")
            for ec in range(DM // 512):
                esl = slice(ec * 512, (ec + 1) * 512)
                ps_out = pso_ref[0].tile([128, 512], F32, tag="pso")
                i = 0
                for ya, wa in ((y8_b, wp8_sb), (ye8_b, wp8_sb),
                               (y8_b, wpe8_sb)):
                    nc.tensor.matmul(ps_out[:],
                                     ya[:, :, tsl],
                                     wa[:, :, esl],
                                     start=(i == 0), stop=(i == 2),
                                     perf_mode=DR)
                    i += 1
                if (tt * 4 + ec) % 5 < 2:
                    nc.vector.tensor_scalar_mul(o_big[:, esl],
                                                ps_out[:], 1.0 / PS)
                else:
                    nc.scalar.mul(o_big[:, esl], ps_out[:], 1.0 / PS)
            r0 = b * T + tt * 128
            nc.sync.dma_start(out[r0:r0 + 128, :], o_big[:])

        def attn_icx(b, icx, y8_b, ye8_b, drain):
            """Generator: one (b, icx) attention chunk, yielding after each
            interleaved jt iteration so callers can weave other work in."""
            i0 = b * T + icx * IC
            jt_hi = (icx + 1) * (IC // 128)
            # slot-0 heads (host-assigned, slope >= 2^-4) decay below
            # e^-48 past ~385 keys: skip those tiles
            jt_lo = [max(0, 4 * icx - JCUT), 0]
            # both heads' row-sum rows live in one PSUM bank
            # (partitions 0 and 32 via matmul tile_position)
            ps_sp = psmisc.tile([33, IC], F32, tag="psm", name="psm")
            ps_sum = [ps_sp[0:1, :], ps_sp[32:33, :]]
            sum_pos = [(0, 0), (0, 32)]
            ps_y = [psacc.tile([128, IC], F32, tag="psy", name=f"psy{hi}")
                    for hi in range(HPC)]
            # the two heads' jt loops are interleaved so PE always has
            # the other head's matmuls during a tile's bias+exp chain
            for jt in range(jt_hi):
                # drain one pending projection row-block per jt iteration:
                # PE gets independent work while this jt's bias+exp chain
                # runs, without starving DVE/ACT
                if drain and pending_proj:
                    emit_proj_tt(*pending_proj.pop(0))
                    # drain faster near the end so the final flush is short
                    if b == 1 and icx == NIC - 1 and pending_proj:
                        emit_proj_tt(*pending_proj.pop(0))
                for hi in range(HPC):
                    if jt < jt_lo[hi]:
                        continue
                    # bf16 matmuls have no small-free-dim penalty, so
                    # clamp the causal offset exactly (128-aligned).
                    o = max(0, jt * 128 - icx * IC)
                    n = IC - o
                    ps_sc = pssc.tile([128, IC], F32, tag="pssc")
                    nc.tensor.matmul(
                        ps_sc[:, o:],
                        k_sb[hi][:, b * T + jt * 128: b * T + (jt + 1) * 128],
                        q_sb[hi][:, i0 + o:i0 + IC],
                        start=True, stop=True)
                    # bias slot s = (it - jt) + 1; slot 1 = diagonal
                    d0 = (icx * IC + o) // 128 - jt + 1
                    ps3 = ps_sc[:, o:].rearrange("p (a c) -> p a c", c=128)
                    # raw biased scores sit near s_raw+alibi with magnitudes
                    # up to ~1e3; stage them in f32r (bf16 ulp there would
                    # distort dominant weights), then exp emits compact bf16
                    # weights.
                    w32 = w32pool.tile([128, IC], F32R, tag="w32")
                    w3 = w32[:, o:].rearrange("p (a c) -> p a c", c=128)
                    nc.vector.scalar_tensor_tensor(
                        out=w3, in0=ps3, scalar=1.0,
                        in1=bias_sb[:, hi, d0:d0 + n // 128, :],
                        op0=MULT, op1=ADD)
                    w_t = wpool2.tile([128, IC], BF16, tag="wt")
                    nc.scalar.activation(w_t[:, o:], w32[:, o:], EXP,
                                         bias=negm[:], scale=1.0 / SQHD)
                    nc.tensor.matmul(ps_y[hi][:, o:],
                                     v_keep[:, b * NT + jt, hi * HD:(hi + 1) * HD],
                                     w_t[:, o:],
                                     start=(jt == jt_lo[hi]), stop=(jt == jt_hi - 1))
                    nc.tensor.matmul(ps_sum[hi][0:1, o:], ones128_sb[:], w_t[:, o:],
                                     start=(jt == jt_lo[hi]), stop=(jt == jt_hi - 1),
                                     tile_position=sum_pos[hi])
                yield
            bcas = []
            for hi in range(HPC):
                recip = smpool.tile([1, IC], F32R, tag=f"recip{hi}",
                                    name=f"recip{hi}")
                with nc.allow_low_precision(reason="f32r is 4-byte"):
                    nc.vector.reciprocal(recip[:], ps_sum[hi][:, :])
                # broadcast recip down 128 partitions on the (idle) gpsimd
                # engine; frees PE of the ones1 matmul and ACT of staging
                bca = smpool.tile([128, IC], F32R, tag=f"bca{hi}",
                                  name=f"bca{hi}")
                nc.gpsimd.partition_broadcast(bca[:], recip[0:1, :])
                bcas.append(bca)
            for hi in range(HPC):
                yf = yfpool.tile([128, IC], F32, tag="yf")
                nc.vector.tensor_mul(yf[:], ps_y[hi][:], bcas[hi][:])
                ics = slice(icx * IC, (icx + 1) * IC)
                # y staged in fp8 at 16x so the residual stays in e4m3's
                # normal range on hardware
                nc.scalar.mul(y8_b[:, hi, ics], yf[:], YS)
                nc.vector.scalar_tensor_tensor(
                    out=ye8_b[:, hi, ics], in0=yf[:], scalar=YS,
                    in1=y8_b[:, hi, ics], op0=MULT, op1=SUB)
            if do3:
                pending_proj.extend(
                    (b, tt, y8_b, ye8_b) for tt in
                    range(icx * (IC // 128), (icx + 1) * (IC // 128)))

        # ---------------- phase 1 (A+B): qkv + rope ----------------
        with tc.tile_pool(name="w1", bufs=1) as wpool, \
             tc.tile_pool(name="xt", bufs=2) as xpool, \
             tc.tile_pool(name="rope", bufs=3) as rpool, \
             tc.tile_pool(name="p1", bufs=2, space="PSUM") as ps1, \
             tc.tile_pool(name="pv", bufs=1, space="PSUM") as psv:
            # 6 weight tensors, each [128, CT, 256] fp8 (4KB/partition)
            wq8_sb = wpool.tile([128, CT, HPC * HD], F8, tag="wq8")
            wqe8_sb = wpool.tile([128, CT, HPC * HD], F8, tag="wqe8")
            wk8_sb = wpool.tile([128, CT, HPC * HD], F8, tag="wk8")
            wke8_sb = wpool.tile([128, CT, HPC * HD], F8, tag="wke8")
            wv8_sb = wpool.tile([128, CT, HPC * HD], F8, tag="wv8")
            wve8_sb = wpool.tile([128, CT, HPC * HD], F8, tag="wve8")

            def xchunk(tchunk):
                """DMA one chunk of x8 and xe8 (one DMA per stream: the SP
                sequencer serializes DMA issue at ~1.3us each)."""
                xa = xpool.tile([128, CT, TCH], F8, tag="x8")
                xb = xpool.tile([128, CT, TCH], F8, tag="xe8")
                # split issue across the two HWDGE queues (SP + ACT): the
                # sequencers serialize DMA issue
                nc.sync.dma_start(xa[:], x8[tchunk])
                nc.scalar.dma_start(xb[:], xe8[tchunk])
                return xa, xb

            # DMA issue order is consumption order: chunk 0's q/k weights and
            # x lead; v weights + phase-2/3 constants trail.
            xa0, xb0 = xchunk(0)
            nc.sync.dma_start(wq8_sb[:], wq8)
            nc.sync.dma_start(wqe8_sb[:], wqe8)
            nc.sync.dma_start(wk8_sb[:], wk8)
            nc.sync.dma_start(wke8_sb[:], wke8)
            nc.sync.dma_start(wv8_sb[:], wv8)
            nc.sync.dma_start(wve8_sb[:], wve8)
            nc.sync.dma_start(cos_sb[:], cosT)
            nc.sync.dma_start(sin_sb[:], sinT)
            xa1, xb1 = xchunk(1)
            nc.sync.dma_start(ones128_sb[:], ones128)
            nc.sync.dma_start(bias_sb[:], biasd)
            nc.sync.dma_start(wp8_sb[:], wp8)
            nc.sync.dma_start(wpe8_sb[:], wpe8)

            def chunk_groups(tchunk, xa, xb, c0=0, cw=TCH):
                """Closures for one chunk's column window [c0, c0+cw):
                q/k tile groups + v tile groups."""
                t0 = tchunk * TCH + c0
                cs = slice(t0 % T, t0 % T + cw)
                xsl = slice(c0, c0 + cw)
                groups = []

                def qk_group(dst, w8_sb, we8_sb, et):
                    ec = slice(et * HD, (et + 1) * HD)
                    ps_q = ps1.tile([128, cw], F32, tag="psq")
                    i = 0
                    for wt, xt in ((w8_sb, xa), (we8_sb, xa), (w8_sb, xb)):
                        for p in range(NP):
                            nc.tensor.matmul(
                                ps_q[:],
                                wt[:, 2 * p:2 * p + 2, ec],
                                xt[:, 2 * p:2 * p + 2, xsl],
                                start=(i == 0), stop=(i == 3 * NP - 1),
                                perf_mode=DR)
                            i += 1
                    qraw = rpool.tile([128, TCH], BF16, tag="qraw")
                    nc.scalar.mul(qraw[:, :cw], ps_q[:], 1.0 / PS)
                    # rotate-half via cross-partition DVE ops; sin_sb rows
                    # 64:128 hold -sin_h, rows 0:64 hold +sin_h. All
                    # operands bf16 => DVE 2x path.
                    tmp = rpool.tile([128, TCH], BF16, tag="tmp")
                    nc.vector.tensor_mul(tmp[0:64, :cw], qraw[64:128, :cw],
                                         sin_sb[64:128, cs])
                    nc.vector.tensor_mul(tmp[64:128, :cw], qraw[0:64, :cw],
                                         sin_sb[0:64, cs])
                    dcols = dst[et][:, t0:t0 + cw]
                    nc.vector.tensor_mul(dcols, qraw[:, :cw], cos_sb[:, cs])
                    nc.vector.tensor_add(dcols, dcols, tmp[:, :cw])

                def v_group(tt, ps_pair, slot):
                    ts = slice(tt * 128, (tt + 1) * 128)
                    i = 0
                    for wt, xt in ((wv8_sb, xa), (wve8_sb, xa), (wv8_sb, xb)):
                        for p in range(NP):
                            nc.tensor.matmul(
                                ps_pair[:, slot, :],
                                xt[:, 2 * p:2 * p + 2, ts],
                                wt[:, 2 * p:2 * p + 2, :],
                                start=(i == 0), stop=(i == 3 * NP - 1),
                                perf_mode=DR)
                            i += 1
                    nc.scalar.mul(v_keep[:, t0 // 128 + tt, :],
                                  ps_pair[:, slot, :], 1.0 / PS)

                for dst, w8_sb, we8_sb in ((q_sb, wq8_sb, wqe8_sb),
                                           (k_sb, wk8_sb, wke8_sb)):
                    for et in range(HPC):
                        groups.append((qk_group, (dst, w8_sb, we8_sb, et)))
                # two v outputs share one PSUM bank (packed halves)
                vstate = {}

                def v_wrap(tt):
                    if vstate.get("pair") is None:
                        vstate["pair"] = psv.tile([128, 2, HPC * HD], F32,
                                                  tag="psv", name="psv")
                        vstate["slot"] = 0
                    v_group(tt, vstate["pair"], vstate["slot"])
                    vstate["slot"] += 1
                    if vstate["slot"] == 2:
                        vstate["pair"] = None

                for tt in range(c0 // 128, (c0 + cw) // 128):
                    groups.append((v_wrap, (tt,)))
                return groups

            # phase A: chunks 0-3 (batch 0 rows), qkv only
            for tchunk in range(4):
                if tchunk == 0:
                    xa, xb = xa0, xb0
                elif tchunk == 1:
                    xa, xb = xa1, xb1
                else:
                    xa, xb = xchunk(tchunk)
                for fn, args in chunk_groups(tchunk, xa, xb):
                    fn(*args)

            # phase B: chunks 4-7 (batch 1 rows) interleaved with batch-0
            # attention (PE-heavy qkv overlaps DVE/ACT-heavy attention);
            # batch-0 projections are deferred to phase C (no free PSUM)
            if do2:
                y8_b0 = ypool.tile([128, HPC, T], F8, tag="y8")
                ye8_b0 = ypool.tile([128, HPC, T], F8, tag="ye8")
            for tchunk in range(4, NCH):
                xa, xb = xchunk(tchunk)
                groups = chunk_groups(tchunk, xa, xb)
                if do2:
                    icx = tchunk - 4
                    feeder = attn_icx(0, icx, y8_b0, ye8_b0, drain=False)
                    jt_hi = (icx + 1) * (IC // 128)
                    done = 0
                    for g, (fn, args) in enumerate(groups):
                        fn(*args)
                        want = ((g + 1) * jt_hi + 7) // 8
                        while done < want and next(feeder, "end") != "end":
                            done += 1
                    for _ in feeder:
                        pass
                else:
                    for fn, args in groups:
                        fn(*args)

        # ---------------- phase C: batch-1 attention + all projections ----
        if do2:
            with tc.tile_pool(name="pso", bufs=3, space="PSUM") as pso:
                pso_ref[0] = pso
                y8_b1 = ypool.tile([128, HPC, T], F8, tag="y8")
                ye8_b1 = ypool.tile([128, HPC, T], F8, tag="ye8")
                for icx in range(NIC):
                    for _ in attn_icx(1, icx, y8_b1, ye8_b1, drain=do3):
                        pass
                for job in pending_proj:
                    emit_proj_tt(*job)

    nc.compile()
    return nc


def _host_tensors():
    """Core-independent constant inputs."""
    inv_freq = 1.0 / (ROPE_THETA ** (np.arange(0, HD, 2, dtype=np.float64) / HD))
    ang = np.arange(T, dtype=np.float64)[:, None] * inv_freq[None, :]   # [T, 64]
    bf16 = mybir.dt.np(BF16)
    cos_h = np.cos(ang).T.astype(np.float32)                            # [64, T]
    sin_h = np.sin(ang).T.astype(np.float32)
    cosT = np.concatenate([cos_h, cos_h], axis=0).astype(bf16)          # [128, T]
    # tmp[0:64] = q[64:128] * sinT[64:128] needs -sin there; tmp[64:128]
    # = q[0:64] * sinT[0:64] needs +sin (halves hold identical angles)
    sinT = np.concatenate([sin_h, -sin_h], axis=0).astype(bf16)

    ones128 = np.ones((128, 1), dtype=bf16)
    return cosT, sinT, ones128


def _bias_tiles(heads):
    """[128, HPC, 17, 128] additive pre-scale ALiBi bias, slot s = (it-jt)+1.

    Slot 0 (it < jt, fully masked) is all NEG; slot 1 (diagonal) has the
    upper triangle NEG; slots 2.. are pure sqrt(HD)*alibi. The -M_OFF
    stability offset is applied as the exp's constant bias so near-diagonal
    entries stay small enough for bf16.
    """
    jj = np.arange(128)[:, None]
    ii = np.arange(128)[None, :]
    rel = (jj - ii).astype(np.float64)          # (jj - ii)
    bias = np.empty((128, HPC, 17, 128), dtype=np.float32)
    for e, h in enumerate(heads):
        slope = 2.0 ** (-8.0 * (h + 1) / H)
        bias[:, e, 0, :] = NEG
        for d in range(16):                      # d = it - jt >= 0
            v = SQHD * slope * (rel - 128.0 * d)
            tile_v = v.astype(np.float32)
            if d == 0:
                tile_v = np.where(jj > ii, NEG, tile_v)
            bias[:, e, d + 1, :] = tile_v
    return bias.astype(mybir.dt.np(BF16))


_NC_CACHE = {}


def _get_program():
    if "nc" not in _NC_CACHE:
        _NC_CACHE["nc"] = build_program()
    return _NC_CACHE["nc"]


def _fp8_split(a, scale):
    """Return (fp8(a*scale), fp8(a*scale - fp8(a*scale))) as e4m3 arrays.

    The scale keeps both the main values and the residuals inside e4m3's
    normal range (subnormals may flush to zero on hardware).
    """
    f8 = mybir.dt.np(F8)
    hi = (a * scale).astype(f8)
    lo = (a * scale - hi.astype(np.float32)).astype(f8)
    return hi, lo


def core_heads(c):
    """Heads owned by core c: a high-slope head (slot 0, ALiBi cutoff
    applies) paired with a low-slope head (slot 1, full attention)."""
    return [c, c + NCORES]


def make_in_maps(x, W_qkv, W_proj):
    x = np.asarray(x, dtype=np.float32)
    W_qkv = np.asarray(W_qkv, dtype=np.float32)
    W_proj = np.asarray(W_proj, dtype=np.float32)

    xT = np.ascontiguousarray(x.reshape(ROWS, DM).T)                # [DM, ROWS]
    x8f, xe8f = _fp8_split(xT, XS)

    def xtile(a):
        # [DM, ROWS] -> [NCH, 128, CT, TCH] (chunk, partition, ktile, t)
        return np.ascontiguousarray(
            a.reshape(CT, 128, NCH, TCH).transpose(2, 1, 0, 3))

    def wtile(a):
        # [DM, E] -> [128, CT, E]
        return np.ascontiguousarray(
            a.reshape(CT, 128, -1).transpose(1, 0, 2))

    def ptile(a):
        # [HPC*HD, DM] -> [128, HPC, DM]
        return np.ascontiguousarray(
            a.reshape(HPC, 128, DM).transpose(1, 0, 2))

    x8, xe8 = xtile(x8f), xtile(xe8f)
    Wq, Wk, Wv = W_qkv[:, :DM], W_qkv[:, DM:2 * DM], W_qkv[:, 2 * DM:]
    cosT, sinT, ones128 = _host_tensors()

    in_maps = []
    for c in range(NCORES):
        ha, hb = core_heads(c)
        cols = np.r_[ha * HD:(ha + 1) * HD, hb * HD:(hb + 1) * HD]
        wq8, wqe8 = _fp8_split(np.ascontiguousarray(Wq[:, cols]), WS)
        wk8, wke8 = _fp8_split(np.ascontiguousarray(Wk[:, cols]), WS)
        wv8, wve8 = _fp8_split(np.ascontiguousarray(Wv[:, cols]), WS)
        wp8, wpe8 = _fp8_split(np.ascontiguousarray(W_proj[cols, :]), WS)
        in_maps.append({
            "x8": x8,
            "xe8": xe8,
            "wq8": wtile(wq8), "wqe8": wtile(wqe8),
            "wk8": wtile(wk8), "wke8": wtile(wke8),
            "wv8": wtile(wv8), "wve8": wtile(wve8),
            "wp8": ptile(wp8), "wpe8": ptile(wpe8),
            "cosT": cosT,
            "sinT": sinT,
            "biasd": _bias_tiles(core_heads(c)),
            "ones128": ones128,
        })
    return in_maps


def kernel(x, causal_mask, W_qkv, W_proj):
    del causal_mask  # always lower-triangular; causality is hardcoded
    nc = _get_program()
    in_maps = make_in_maps(x, W_qkv, W_proj)
    res = run_bass_kernel_spmd(nc, in_maps, core_ids=list(range(NCORES)))
    acc = np.zeros((ROWS, DM), dtype=np.float32)
    for c in range(NCORES):
        acc += np.asarray(res.results[c]["out"], dtype=np.float32)
    return acc.reshape(B, T, DM)
